# revision 1
# baseline (speedup 1.0000x reference)
"""Trainium2 Bass kernel for nn_Attention_14542759264705.

Dense transformer attention: QKV proj + interleaved RoPE + GQA causal
attention (32 q heads / 8 kv heads, hd=64) + output proj, fp32 in/out.

Sharding: tensor-parallel over kv-head groups across 8 cores. Core c owns
q heads 4c..4c+3 and kv head c; each core computes a partial output and
the host sums the 8 partials.

v2 (vs the fp32r baseline):
  - x is transposed on the HOST (xT input) -> no PE transposes / ACT
    copies for the projection's moving operand.
  - All matmul operands are bf16 (PSUM accumulation stays f32): halves
    DMA volume, enables fast weight load, 2x DVE on 16-bit tiles. fp32r
    at N>=256 is already 1 cyc/row, so MM time is unchanged; the wins
    are bandwidth + LDWEIGHTS + elementwise.
  - The two per-head-pair exps are merged into one [128,1024] ACT call
    (amortizes the ~352-cycle ACT fixed cost; ACT exp is the phase-2
    pace-setter).
  - Softmax normalization: DVE reciprocal_approx_fast on the [1,1024]
    denominator row (the old nc.vector.reciprocal was 8 cyc/elem),
    gpsimd partition-broadcast, DVE muls. All off the PE critical path.
  - wo matmuls are drip-fed one (qs,do) step per kt-iteration into the
    NEXT pair's attention loop so the PE never parks while ACT works,
    instead of a 7us wo burst that starves ACT.
"""
import numpy as np

B, S, D = 2, 2048, 2048
T = B * S
NH, NKV, HD = 32, 8, 64
NCORES = 8

_cache = {}


def _build(phases=99):
    from collections import deque

    import concourse.bacc as bacc
    import concourse.mybir as mybir
    import concourse.tile as tile
    from concourse.masks import make_identity

    F32 = mybir.dt.float32
    BF16 = mybir.dt.bfloat16
    AF = mybir.ActivationFunctionType

    # Force Exp/Ln/Copy onto the single combined act table set so the
    # compiler never inserts per-call ACT_TABLE_LOADs between the phase-2
    # exps and the Ln/Exp reciprocal (keeps act_func_set ids truthful:
    # dict order is unchanged, other sets just lose the overlapping funcs).
    from concourse.hw_specs import get_activation_tables as _gat

    def _patched_tables(arch):
        tabs = _gat(arch)
        key = "natural_log_exp_and_others"
        comb = tabs[key]
        return {n: (s if n == key else (s - comb)) for n, s in tabs.items()}

    _orig_gat = bacc.get_activation_tables
    bacc.get_activation_tables = _patched_tables

    nc = bacc.Bacc("TRN2", target_bir_lowering=False, debug=False,
                   num_devices=NCORES)
    xT = nc.dram_tensor("xT", [D, T], BF16, kind="ExternalInput").ap()
    wqkvT = nc.dram_tensor("wqkvT", [D, 384], BF16, kind="ExternalInput").ap()
    woT = nc.dram_tensor("woT", [256, D], BF16, kind="ExternalInput").ap()
    c4 = nc.dram_tensor("c4", [128, S], BF16, kind="ExternalInput").ap()
    s4 = nc.dram_tensor("s4", [128, S], BF16, kind="ExternalInput").ap()
    maskP = nc.dram_tensor("maskP", [128, 2 * 4 * 512], BF16,
                           kind="ExternalInput").ap()
    o = nc.dram_tensor("o", [T, D], BF16, kind="ExternalOutput").ap()

    with tile.TileContext(nc) as tc:
        with tc.tile_pool(name="resident", bufs=1) as res:
            ident64 = res.tile([64, 64], BF16)
            make_identity(nc, ident64[:])
            c4_sb = res.tile([128, S], BF16)
            s4_sb = res.tile([128, S], BF16)
            maskP_sb = res.tile([128, 2 * 4 * 512], BF16)

            QRI_A = res.tile([128, T], BF16)   # [h0r h0i h1r h1i] x tokens
            QRI_B = res.tile([128, T], BF16)   # [h2r h2i h3r h3i]
            KRI2 = res.tile([128, T], BF16)    # [Kr Ki Kr Ki]
            Vt_sb = res.tile([128, 32 * 65], BF16)  # kt-tile k at cols k*65
            Vt3 = Vt_sb.rearrange("p (k c) -> p k c", c=65)
            wqkv_r = res.tile([128, 16 * 384], BF16)
            woT_r = res.tile([128, 2 * D], BF16)
            ones32 = res.tile([128, 32], BF16)
            nc.gpsimd.memset(ones32[:], 1.0)
            nc.vector.tensor_copy(Vt3[:, :, 64], ones32[:])
            dbg_pg = (res.tile([128, 1024], BF16, name="dbg_pg")
                      if phases == 3 else None)


            # ---------------- phase 1: xT DMA, proj, rope -------------------
            with tc.tile_pool(name="xtp", bufs=3) as xtp, \
                 tc.tile_pool(name="ropet", bufs=2) as rp, \
                 tc.tile_pool(name="vtps", bufs=1, space="PSUM") as vtp_pool, \
                 tc.tile_pool(name="projps", bufs=1, space="PSUM") as projp:

                vsb_prev = None

                def emit_vt(jj, vsb):
                    vtp = vtp_pool.tile([128, 256], BF16, name="vtp")
                    for i in range(4):
                        nc.tensor.transpose(
                            vtp[:, i * 64:(i + 1) * 64],
                            vsb[:, i * 128:(i + 1) * 128], ident64[:])
                    vtp3 = vtp.rearrange("p (k c) -> p k c", c=64)
                    nc.vector.tensor_copy(
                        Vt3[:, jj * 4:jj * 4 + 4, 0:64], vtp3[:])

                for j in range(8):           # 512-token chunks
                    xts = []
                    for d in range(16):
                        xt = xtp.tile([128, 512], BF16, name=f"xt{d}")
                        nc.sync.dma_start(
                            xt[:], xT[d * 128:(d + 1) * 128,
                                      j * 512:(j + 1) * 512])
                        xts.append(xt)
                        if j == 0:
                            # interleave so MM(d) unblocks asap at startup
                            nc.sync.dma_start(
                                wqkv_r[:, d * 384:(d + 1) * 384],
                                wqkvT[d * 128:(d + 1) * 128, :])
                    if j == 0:
                        nc.sync.dma_start(c4_sb[:], c4[:])
                        nc.sync.dma_start(s4_sb[:], s4[:])
                    elif j == 1:
                        # phase-2-only tensors, off the startup critical path
                        nc.sync.dma_start(maskP_sb[:], maskP[:])
                        for t in range(2):
                            nc.sync.dma_start(woT_r[:, t * D:(t + 1) * D],
                                              woT[t * 128:(t + 1) * 128, :])
                    QRp = projp.tile([128, 512], F32, name="QRp", bufs=2)
                    QIp = projp.tile([128, 512], F32, name="QIp", bufs=2)
                    KVp = projp.tile([128, 512], F32, name="KVp", bufs=2)
                    for d in range(16):
                        for ch, ps in enumerate((QRp, QIp, KVp)):
                            nc.tensor.matmul(
                                ps[:],
                                wqkv_r[:, d * 384 + ch * 128:
                                       d * 384 + (ch + 1) * 128],
                                xts[d][:], start=(d == 0), stop=(d == 15))
                        if d == 8 and vsb_prev is not None:
                            emit_vt(j - 1, vsb_prev)
                    # rope: ACT stages psum->bf16 SBUF, DVE does bf16 TT
                    # at 2x; KVp released by the kvb copy immediately.
                    tb = j * 512
                    bc = (j % 4) * 512
                    cs = c4_sb[:, bc:bc + 512]
                    sn = s4_sb[:, bc:bc + 512]
                    cs32 = c4_sb[0:32, bc:bc + 512]
                    sn32 = s4_sb[0:32, bc:bc + 512]
                    kb = rp.tile([64, 512], BF16, name="kb")
                    vsb = rp.tile([64, 512], BF16, name="vsb")
                    qrb = rp.tile([128, 512], BF16, name="qrb")
                    qib = rp.tile([128, 512], BF16, name="qib")
                    nc.scalar.copy(kb[:], KVp[0:64, :])
                    nc.scalar.copy(vsb[:], KVp[64:128, :])
                    nc.scalar.copy(qrb[:], QRp[:])
                    nc.scalar.copy(qib[:], QIp[:])
                    u1 = rp.tile([32, 512], BF16, name="u1", bufs=1)
                    u2 = rp.tile([32, 512], BF16, name="u2", bufs=1)
                    u3 = rp.tile([32, 512], BF16, name="u3", bufs=1)
                    u4 = rp.tile([32, 512], BF16, name="u4", bufs=1)
                    cs32b = c4_sb[32:64, bc:bc + 512]
                    sn32b = s4_sb[32:64, bc:bc + 512]
                    nc.vector.tensor_mul(u1[:], kb[0:32, :], cs32)
                    nc.vector.tensor_mul(u2[:], kb[32:64, :], sn32b)
                    nc.vector.tensor_mul(u3[:], kb[0:32, :], sn32)
                    nc.vector.tensor_mul(u4[:], kb[32:64, :], cs32b)
                    for g in (0, 64):
                        nc.vector.tensor_sub(
                            KRI2[g:g + 32, tb:tb + 512], u1[:], u2[:])
                    for g in (32, 96):
                        nc.vector.tensor_add(
                            KRI2[g:g + 32, tb:tb + 512], u3[:], u4[:])
                    t1 = rp.tile([128, 512], BF16, name="t1", bufs=1)
                    t2 = rp.tile([128, 512], BF16, name="t2", bufs=1)
                    t3 = rp.tile([128, 512], BF16, name="t3", bufs=1)
                    t4 = rp.tile([128, 512], BF16, name="t4", bufs=1)
                    nc.vector.tensor_mul(t1[:], qrb[:], cs)
                    nc.vector.tensor_mul(t3[:], qrb[:], sn)
                    nc.vector.tensor_mul(t2[:], qib[:], sn)
                    nc.vector.tensor_mul(t4[:], qib[:], cs)
                    for hh in range(4):
                        dst = QRI_A if hh < 2 else QRI_B
                        base = (hh % 2) * 64
                        nc.vector.tensor_sub(
                            dst[base:base + 32, tb:tb + 512],
                            t1[32 * hh:32 * hh + 32, :],
                            t2[32 * hh:32 * hh + 32, :])
                        nc.vector.tensor_add(
                            dst[base + 32:base + 64, tb:tb + 512],
                            t3[32 * hh:32 * hh + 32, :],
                            t4[32 * hh:32 * hh + 32, :])
                    vsb_prev = vsb
                emit_vt(7, vsb_prev)

            dbg = {}
            if phases < 2:
                nc.sync.dma_start(o[0:128, :], QRI_A[:, 0:2048])
            # -------------- phase 2: attention + wo, per qt-512 pair --------
            else:
                with tc.tile_pool(name="probs", bufs=4) as probsp, \
                     tc.tile_pool(name="attnp", bufs=2) as attnp, \
                     tc.tile_pool(name="normp", bufs=2) as normp, \
                     tc.tile_pool(name="outp", bufs=2) as outp, \
                     tc.tile_pool(name="sps", bufs=2, space="PSUM") as sps, \
                     tc.tile_pool(name="pvps", bufs=1, space="PSUM") as pvps, \
                     tc.tile_pool(name="ops", bufs=2, space="PSUM") as opsp:

                    pending = deque()
                    norm_q = deque()
                    nprog = {"enq": 0, "done": 0}

                    def enqueue_wo(attn01, attn23, qb):
                        state = {}
                        pid = nprog["enq"]
                        nprog["enq"] += 1
                        for qs in range(4):
                            for do in range(4):
                                pending.append(
                                    (pid, attn01, attn23, qb, qs, do, state))

                    def drain_wo(n):
                        for _ in range(n):
                            if not pending:
                                return
                            if pending[0][0] >= nprog["done"]:
                                return   # this pair's attn not normed yet
                            pid, attn01, attn23, qb, qs, do, state = \
                                pending.popleft()
                            if do == 0:
                                state[qs] = outp.tile([128, D], BF16,
                                                      name="osb")
                            osb = state[qs]
                            Ops = opsp.tile([128, 512], F32, name="Ops")
                            nc.tensor.matmul(
                                Ops[:], attn01[:, qs * 128:(qs + 1) * 128],
                                woT_r[:, do * 512:(do + 1) * 512],
                                start=True, stop=False)
                            nc.tensor.matmul(
                                Ops[:], attn23[:, qs * 128:(qs + 1) * 128],
                                woT_r[:, D + do * 512:D + (do + 1) * 512],
                                start=False, stop=True)
                            nc.vector.tensor_copy(
                                osb[:, do * 512:(do + 1) * 512], Ops[:])
                            if do == 3 and phases != 3:
                                qq = qb + qs * 128
                                nc.sync.dma_start(o[qq:qq + 128, :], osb[:])

                    def emit_pv(PVs, b, pkt, ppg, pcsl, stop):
                        vt = Vt3[:, b * 16 + pkt, :]
                        st = (pkt == 0)
                        for hh in range(2):
                            hs = hh * 512
                            nc.tensor.matmul(
                                PVs[:, hs + pcsl.start:hs + pcsl.stop], vt,
                                ppg[:, hs + pcsl.start:hs + pcsl.stop],
                                start=st, stop=stop)

                    for b in range(2):
                        for jp in range(4):          # qt-512 pairs
                            qb = b * S + jp * 512
                            nkt = 4 * jp + 4
                            attn01 = attnp.tile([128, 512], BF16, name="at01")
                            attn23 = attnp.tile([128, 512], BF16, name="at23")
                            pvc = normp.tile([65, 2048], F32, name="pvc")
                            for pi, (QRI, attn) in enumerate(
                                    ((QRI_A, attn01), (QRI_B, attn23))):
                                PVs = pvps.tile([65, 1024], F32, name="PV")
                                pgq = deque()
                                for kt in range(nkt):
                                    kc = b * S + kt * 128
                                    r = kt - (nkt - 4)
                                    half = r >= 2   # only right half live
                                    csl = slice(256, 512) if half \
                                        else slice(0, 512)
                                    Sg = sps.tile([128, 1024], F32, name="Sg")
                                    pg = probsp.tile([128, 1024], BF16,
                                                     name="pg")
                                    for hh in range(2):
                                        hs = hh * 512
                                        nc.tensor.matmul(
                                            Sg[:, hs + csl.start:
                                               hs + csl.stop],
                                            KRI2[64 * hh:64 * hh + 64,
                                                 kc:kc + 128],
                                            QRI[64 * hh:64 * hh + 64,
                                                qb + csl.start:
                                                qb + csl.stop],
                                            start=True, stop=True,
                                            tile_position=(64 * hh, 0))
                                    if half:
                                        sgv = Sg.rearrange(
                                            "p (h c) -> p h c",
                                            h=2)[:, :, 256:512]
                                        pgv = pg.rearrange(
                                            "p (h c) -> p h c",
                                            h=2)[:, :, 256:512]
                                        nc.scalar.activation(
                                            pgv, sgv, AF.Exp, scale=0.125)
                                    else:
                                        nc.scalar.activation(
                                            pg[:], Sg[:], AF.Exp, scale=0.125)
                                    if r >= 0:
                                        if half:
                                            msl = slice(512 * r + 256,
                                                        512 * r + 512)
                                            psl = slice(256, 512)
                                        else:
                                            msl = slice(512 * r,
                                                        512 * r + 256)
                                            psl = slice(0, 256)
                                        pgv = pg.rearrange(
                                            "p (h c) -> p h c", h=2)[:, :, psl]
                                        mkv = maskP_sb.rearrange(
                                            "p (h c) -> p h c", h=2)[:, :, msl]
                                        nc.vector.tensor_mul(pgv, pgv, mkv)
                                    pgq.append((kt, pg, csl))
                                    if len(pgq) > 2:
                                        emit_pv(PVs, b, *pgq.popleft(),
                                                stop=False)
                                    if (phases == 3 and b == 1 and jp == 3
                                            and pi == 1 and kt == nkt - 4):
                                        nc.vector.tensor_copy(dbg_pg[:],
                                                              pg[:])
                                    if pi == 0 and kt in (1, 2, 3) and norm_q:
                                        norm_q.popleft()()
                                    drain_wo(2 if len(pending) > 16 else 1)
                                while pgq:
                                    item = pgq.popleft()
                                    emit_pv(PVs, b, *item, stop=not pgq)
                                # free the PV banks asap; norm happens once
                                # per pair, off the PE critical path
                                nc.vector.tensor_copy(
                                    pvc[:, 1024 * pi:1024 * pi + 1024],
                                    PVs[:])
                            # norm stages are deferred into the NEXT pair's
                            # kt loop so the ACT FIFO never stalls on them
                            def make_norm(pvc, attn01, attn23, last):
                                st = {}

                                def s1():
                                    st["lnd"] = normp.tile(
                                        [1, 2048], F32, name="lnd")
                                    nc.scalar.activation(
                                        st["lnd"][:], pvc[64:65, :], AF.Ln)

                                def s2():
                                    st["rec"] = normp.tile(
                                        [1, 2048], F32, name="rec")
                                    nc.scalar.activation(
                                        st["rec"][:], st["lnd"][:],
                                        AF.Exp, scale=-1.0)

                                def s3():
                                    bcst = normp.tile(
                                        [64, 2048], F32, name="bc")
                                    nc.gpsimd.partition_broadcast(
                                        bcst[:], st["rec"][:])
                                    for pi, attn in enumerate(
                                            (attn01, attn23)):
                                        for hh in range(2):
                                            cs_ = 1024 * pi + 512 * hh
                                            nc.vector.tensor_mul(
                                                attn[64 * hh:
                                                     64 * hh + 64, :],
                                                pvc[0:64, cs_:cs_ + 512],
                                                bcst[:, cs_:cs_ + 512])
                                    nprog["done"] += 1
                                    if last:
                                        dbg.update(pvc=pvc, rec=st["rec"],
                                                   bcst=bcst)
                                return [s1, s2, s3]

                            norm_q.extend(make_norm(
                                pvc, attn01, attn23, b == 1 and jp == 3))
                            enqueue_wo(attn01, attn23, qb)
                            if b == 1 and jp == 3:
                                dbg.update(at01=attn01, at23=attn23)
                    while norm_q:
                        norm_q.popleft()()
                    drain_wo(1 << 30)
                    if phases == 3:
                        # debug dumps into sacrificial o rows (bf16)
                        scr = normp.tile([128, 2048], BF16, name="dscr")
                        nc.gpsimd.memset(scr[:], 0.0)
                        nc.sync.dma_start(o[0:128, :], QRI_A[:, 0:2048])
                        nc.sync.dma_start(o[128:256, :], KRI2[:, 0:2048])
                        nc.sync.dma_start(o[256:384, :], Vt_sb[:, 0:2048])
                        nc.vector.tensor_copy(scr[0:65, :], dbg["pvc"][:])
                        nc.sync.dma_start(o[384:512, :], scr[:])
                        scr4 = normp.tile([64, 2048], BF16, name="dscr4")
                        nc.vector.tensor_copy(scr4[:], dbg["bcst"][:])
                        nc.sync.dma_start(o[768:832, :], scr4[:])
                        nc.sync.dma_start(o[512:640, 0:1024], dbg_pg[:])
                        scr3 = normp.tile([128, 2048], BF16, name="dscr3")
                        nc.vector.tensor_copy(scr3[:, 0:512], dbg["at01"][:])
                        nc.vector.tensor_copy(scr3[:, 512:1024],
                                              dbg["at23"][:])
                        nc.sync.dma_start(o[640:768, 0:1024],
                                          scr3[:, 0:1024])

    nc.compile()
    bacc.get_activation_tables = _orig_gat
    return nc


def _prep_inputs(x, freqs_cos, freqs_sin, wq, wk, wv, wo):
    from ml_dtypes import bfloat16
    xf = np.asarray(x, np.float32).reshape(T, D)
    xTf = np.ascontiguousarray(xf.T).astype(bfloat16)      # [D, T]
    wq = np.asarray(wq, np.float32)
    wk = np.asarray(wk, np.float32)
    wv = np.asarray(wv, np.float32)
    wo = np.asarray(wo, np.float32)
    fc = np.asarray(freqs_cos, np.float32)
    fs = np.asarray(freqs_sin, np.float32)

    c4 = np.ascontiguousarray(np.tile(fc.T, (4, 1))).astype(bfloat16)
    s4 = np.ascontiguousarray(np.tile(fs.T, (4, 1))).astype(bfloat16)
    kt = np.arange(128)[:, None]
    qt = np.arange(256)[None, :]
    mA = (kt <= qt).astype(np.float32)
    mB = (kt + 128 <= qt).astype(np.float32)
    one = np.ones((128, 256), np.float32)
    zero = np.zeros((128, 256), np.float32)
    maskP1 = np.concatenate([
        np.concatenate([mA, one], axis=1),
        np.concatenate([mB, one], axis=1),
        np.concatenate([zero, mA], axis=1),
        np.concatenate([zero, mB], axis=1)], axis=1)
    maskP = np.ascontiguousarray(
        np.tile(maskP1, (1, 2))).astype(bfloat16)      # [128, 4096]
    ev = np.arange(0, 64, 2)
    od = np.arange(1, 64, 2)

    in_maps = []
    for c in range(NCORES):
        qreal = np.concatenate([(4 * c + h) * 64 + ev for h in range(4)])
        qimag = np.concatenate([(4 * c + h) * 64 + od for h in range(4)])
        Wc = np.concatenate([wq[qreal], wq[qimag], wk[c * 64 + ev],
                             wk[c * 64 + od], wv[c * 64:(c + 1) * 64]], axis=0)
        in_maps.append({
            "xT": xTf,
            "wqkvT": np.ascontiguousarray(Wc.T).astype(bfloat16),
            "woT": np.ascontiguousarray(
                wo[:, c * 256:(c + 1) * 256].T).astype(bfloat16),
            "c4": c4, "s4": s4, "maskP": maskP,
        })
    return in_maps


def _run(in_maps, trace=False, **kw):
    from concourse import bass_utils
    if "nc" not in _cache:
        _cache["nc"] = _build()
    return bass_utils.run_bass_kernel_spmd(
        _cache["nc"], in_maps, core_ids=list(range(NCORES)), trace=trace, **kw)


def kernel(x, freqs_cos, freqs_sin, wq, wk, wv, wo):
    in_maps = _prep_inputs(x, freqs_cos, freqs_sin, wq, wk, wv, wo)
    res = _run(in_maps)
    out = np.zeros((T, D), np.float64)
    for c in range(NCORES):
        out += np.asarray(res.results[c]["o"], np.float32)
    return out.astype(np.float32).reshape(B, S, D)



# revision 5
# speedup vs baseline: 1.2544x; 1.2544x over previous
"""Trainium2 Bass kernel for nn_Attention_14542759264705.

Dense transformer attention: QKV proj + interleaved RoPE + GQA causal
attention (32 q heads / 8 kv heads, hd=64) + output proj, fp32 in/out.

Sharding: tensor-parallel over kv-head groups across 8 cores. Core c owns
q heads 4c..4c+3 and kv head c; each core computes a partial output and
the host sums the 8 partials.

v3 (vs the v2 two-phase baseline):
  - Single fused PE stream: projection j-chunks are interleaved with
    attention pairs (proj0, proj1, A00, proj2, A01, ... A13) so the PE
    never parks at phase/pair boundaries (v2 lost ~146us to HAM
    re-throttling during starvation windows).
  - Warm-up matmuls at t=0 (no DMA deps) lift the PE clock gate before
    the first real matmul's data lands.
  - Dual-queue DMA: x tiles + weights alternate between the Sync and
    Scalar hardware-DGE queues (~2x ingest); x is tile-contiguous in
    DRAM so each [128,512] tile is one linear 128KB read.
  - Causal trim at 128-col granularity (v2 trimmed at 256) for scores,
    exp, and PV; tri-mask multiply is a single [128,2,128] DVE op.
  - Softmax denominators: [1,2048] row is DMA-reshaped to [16,128], so
    the Ln/Exp reciprocal costs ~0.7us of ACT instead of 4.6us.
  - wo psum->sbuf casts alternate DVE/ACT to split the ~100us cast load.
  - Projection channels run sequentially through one [128,512] psum slot
    pool (shared with wo + V-transposes) so proj+attention coexist in
    the 8 PSUM banks.
"""
import numpy as np

B, S, D = 2, 2048, 2048
T = B * S
NH, NKV, HD = 32, 8, 64
NCORES = 8

_cache = {}


def _build():
    from collections import deque
    from itertools import cycle

    import concourse.bacc as bacc
    import concourse.mybir as mybir
    import concourse.tile as tile
    from concourse.masks import make_identity

    F32 = mybir.dt.float32
    BF16 = mybir.dt.bfloat16
    AF = mybir.ActivationFunctionType

    # Force Exp/Ln/Copy onto the single combined act table set so the
    # compiler never inserts per-call ACT_TABLE_LOADs between exps and
    # the Ln/Exp reciprocal.
    from concourse.hw_specs import get_activation_tables as _gat

    def _patched_tables(arch):
        tabs = _gat(arch)
        key = "natural_log_exp_and_others"
        comb = tabs[key]
        return {n: (s if n == key else (s - comb)) for n, s in tabs.items()}

    _orig_gat = bacc.get_activation_tables
    bacc.get_activation_tables = _patched_tables

    nc = bacc.Bacc("TRN2", target_bir_lowering=False, debug=False,
                   num_devices=NCORES)
    # x tiles stored tile-contiguous: tile (d, j) at rows (d*8+j)*128
    xTt = nc.dram_tensor("xTt", [16 * 8 * 128, 512], BF16,
                         kind="ExternalInput").ap()
    wqkvT = nc.dram_tensor("wqkvT", [D, 384], BF16, kind="ExternalInput").ap()
    woT = nc.dram_tensor("woT", [256, D], BF16, kind="ExternalInput").ap()
    c4 = nc.dram_tensor("c4", [128, S], BF16, kind="ExternalInput").ap()
    s4 = nc.dram_tensor("s4", [128, S], BF16, kind="ExternalInput").ap()
    maskT = nc.dram_tensor("maskT", [128, 256], BF16,
                           kind="ExternalInput").ap()
    o = nc.dram_tensor("o", [T, D], BF16, kind="ExternalOutput").ap()

    with tile.TileContext(nc) as tc:
        with tc.tile_pool(name="res", bufs=1) as res, \
             tc.tile_pool(name="xtp", bufs=2) as xtp, \
             tc.tile_pool(name="ropet", bufs=2) as rp, \
             tc.tile_pool(name="probs", bufs=4) as probsp, \
             tc.tile_pool(name="attnp", bufs=2) as attnp, \
             tc.tile_pool(name="normp", bufs=2) as normp, \
             tc.tile_pool(name="outp", bufs=2) as outp, \
             tc.tile_pool(name="mmps", bufs=2, space="PSUM") as mmps, \
             tc.tile_pool(name="sps", bufs=2, space="PSUM") as sps, \
             tc.tile_pool(name="pvps", bufs=1, space="PSUM") as pvps:

            ident64 = res.tile([64, 64], BF16)
            make_identity(nc, ident64[:])
            c4_sb = res.tile([128, S], BF16)
            s4_sb = res.tile([128, S], BF16)
            maskT_sb = res.tile([128, 256], BF16)
            maskT3 = maskT_sb.rearrange("p (h c) -> p h c", c=128)

            QRI_A = res.tile([128, T], BF16)   # [h0r h0i h1r h1i] x tokens
            QRI_B = res.tile([128, T], BF16)   # [h2r h2i h3r h3i]
            KRI2 = res.tile([128, T], BF16)    # [Kr Ki Kr Ki]
            Vt_sb = res.tile([128, 32 * 65], BF16)  # kt-tile k at cols k*65
            Vt3 = Vt_sb.rearrange("p (k c) -> p k c", c=65)
            wqkv_r = res.tile([128, 16 * 384], BF16)
            woT_r = res.tile([128, 2 * D], BF16)
            ones32 = res.tile([128, 32], BF16)
            nc.gpsimd.memset(ones32[:], 1.0)
            nc.vector.tensor_copy(Vt3[:, :, 64], ones32[:])
            warmM = res.tile([128, 512], BF16)
            nc.gpsimd.memset(warmM[:], 0.0)

            # ---- PE warm-up: lift the HAM clock gate before data lands
            for _ in range(10):
                wps = sps.tile([128, 1024], F32, name="Sg")
                nc.tensor.matmul(wps[:, 0:512], warmM[:, 0:128], warmM[:],
                                 start=True, stop=True)

            # ---- DMA issue: alternate hardware-DGE queues (Sync, Scalar)
            qcyc = cycle((nc.sync, nc.scalar))

            def dma(dst, src):
                next(qcyc).dma_start(dst, src)

            xt_tiles = {}

            def issue_xt(j):
                xts = []
                for d in range(16):
                    xt = xtp.tile([128, 512], BF16, name=f"xt{d}")
                    dma(xt[:], xTt[(d * 8 + j) * 128:(d * 8 + j + 1) * 128, :])
                    xts.append(xt)
                    if j == 0:
                        dma(wqkv_r[:, d * 384:(d + 1) * 384],
                            wqkvT[d * 128:(d + 1) * 128, :])
                xt_tiles[j] = xts
                if j == 0:
                    dma(c4_sb[:], c4[:])
                    dma(s4_sb[:], s4[:])
                elif j == 1:
                    dma(maskT_sb[:], maskT[:])
                    for t in range(2):
                        dma(woT_r[:, t * D:(t + 1) * D],
                            woT[t * 128:(t + 1) * 128, :])

            issue_xt(0)
            issue_xt(1)

            # ---------------- projection + rope for one 512-token chunk
            def proj(j):
                xts = xt_tiles.pop(j)
                kb = rp.tile([64, 512], BF16, name="kb")
                vsb = rp.tile([64, 512], BF16, name="vsb")
                qrb = rp.tile([128, 512], BF16, name="qrb")
                qib = rp.tile([128, 512], BF16, name="qib")
                for ch in range(3):
                    ps = mmps.tile([128, 512], F32, name="mm")
                    for d in range(16):
                        nc.tensor.matmul(
                            ps[:],
                            wqkv_r[:, d * 384 + ch * 128:
                                   d * 384 + (ch + 1) * 128],
                            xts[d][:], start=(d == 0), stop=(d == 15))
                    # stage before the next tile() call recycles the slot
                    if ch == 0:
                        nc.scalar.copy(qrb[:], ps[:])
                    elif ch == 1:
                        nc.scalar.copy(qib[:], ps[:])
                    else:
                        nc.scalar.copy(kb[:], ps[0:64, :])
                        nc.scalar.copy(vsb[:], ps[64:128, :])
                tb = j * 512
                bc = (j % 4) * 512
                cs = c4_sb[:, bc:bc + 512]
                sn = s4_sb[:, bc:bc + 512]
                cs32 = c4_sb[0:32, bc:bc + 512]
                sn32 = s4_sb[0:32, bc:bc + 512]
                u1 = rp.tile([32, 512], BF16, name="u1", bufs=1)
                u2 = rp.tile([32, 512], BF16, name="u2", bufs=1)
                u3 = rp.tile([32, 512], BF16, name="u3", bufs=1)
                u4 = rp.tile([32, 512], BF16, name="u4", bufs=1)
                cs32b = c4_sb[32:64, bc:bc + 512]
                sn32b = s4_sb[32:64, bc:bc + 512]
                nc.vector.tensor_mul(u1[:], kb[0:32, :], cs32)
                nc.vector.tensor_mul(u2[:], kb[32:64, :], sn32b)
                nc.vector.tensor_mul(u3[:], kb[0:32, :], sn32)
                nc.vector.tensor_mul(u4[:], kb[32:64, :], cs32b)
                for g in (0, 64):
                    nc.vector.tensor_sub(
                        KRI2[g:g + 32, tb:tb + 512], u1[:], u2[:])
                for g in (32, 96):
                    nc.vector.tensor_add(
                        KRI2[g:g + 32, tb:tb + 512], u3[:], u4[:])
                t1 = rp.tile([128, 512], BF16, name="t1", bufs=1)
                t2 = rp.tile([128, 512], BF16, name="t2", bufs=1)
                t3 = rp.tile([128, 512], BF16, name="t3", bufs=1)
                t4 = rp.tile([128, 512], BF16, name="t4", bufs=1)
                nc.vector.tensor_mul(t1[:], qrb[:], cs)
                nc.vector.tensor_mul(t3[:], qrb[:], sn)
                nc.vector.tensor_mul(t2[:], qib[:], sn)
                nc.vector.tensor_mul(t4[:], qib[:], cs)
                for hh in range(4):
                    dst = QRI_A if hh < 2 else QRI_B
                    base = (hh % 2) * 64
                    nc.vector.tensor_sub(
                        dst[base:base + 32, tb:tb + 512],
                        t1[32 * hh:32 * hh + 32, :],
                        t2[32 * hh:32 * hh + 32, :])
                    nc.vector.tensor_add(
                        dst[base + 32:base + 64, tb:tb + 512],
                        t3[32 * hh:32 * hh + 32, :],
                        t4[32 * hh:32 * hh + 32, :])
                # V transpose tiles (PE, psum slot shared with proj/wo)
                vtp = mmps.tile([128, 256], BF16, name="vtp", tag="mm")
                for i in range(4):
                    nc.tensor.transpose(
                        vtp[:, i * 64:(i + 1) * 64],
                        vsb[:, i * 128:(i + 1) * 128], ident64[:])
                vtp3 = vtp.rearrange("p (k c) -> p k c", c=64)
                nc.vector.tensor_copy(
                    Vt3[:, j * 4:j * 4 + 4, 0:64], vtp3[:])

            # ---------------- attention machinery
            pending = deque()
            norm_q = deque()
            nprog = {"enq": 0, "done": 0}

            def enqueue_wo(attn01, attn23, qb):
                state = {}
                pid = nprog["enq"]
                nprog["enq"] += 1
                for qs in range(4):
                    for do in range(4):
                        pending.append(
                            (pid, attn01, attn23, qb, qs, do, state))

            def drain_wo(n):
                for _ in range(n):
                    if not pending:
                        return
                    if pending[0][0] >= nprog["done"]:
                        return   # this pair's attn not normed yet
                    pid, attn01, attn23, qb, qs, do, state = \
                        pending.popleft()
                    if do == 0:
                        state[qs] = outp.tile([128, D], BF16, name="osb")
                    osb = state[qs]
                    Ops = mmps.tile([128, 512], F32, name="mm")
                    nc.tensor.matmul(
                        Ops[:], attn01[:, qs * 128:(qs + 1) * 128],
                        woT_r[:, do * 512:(do + 1) * 512],
                        start=True, stop=False)
                    nc.tensor.matmul(
                        Ops[:], attn23[:, qs * 128:(qs + 1) * 128],
                        woT_r[:, D + do * 512:D + (do + 1) * 512],
                        start=False, stop=True)
                    nc.vector.tensor_copy(
                        osb[:, do * 512:(do + 1) * 512], Ops[:])
                    if do == 3:
                        qq = qb + qs * 128
                        nc.sync.dma_start(o[qq:qq + 128, :], osb[:])

            def emit_pv(PVs, b, pkt, ppg, pcsl, stop):
                vt = Vt3[:, b * 16 + pkt, :]
                st = (pkt == 0)
                for hh in range(2):
                    hs = hh * 512
                    nc.tensor.matmul(
                        PVs[:, hs + pcsl.start:hs + pcsl.stop], vt,
                        ppg[:, hs + pcsl.start:hs + pcsl.stop],
                        start=st, stop=stop)

            def make_norm(pvc, attn01, attn23):
                st = {}

                def s1():
                    st["d16"] = normp.tile([16, 128], F32, name="d16")
                    nc.sync.dma_start(st["d16"][:], pvc[64:65, :])

                def s2():
                    l16 = normp.tile([16, 128], F32, name="l16")
                    nc.scalar.activation(l16[:], st["d16"][:], AF.Ln)
                    st["r16"] = normp.tile([16, 128], F32, name="r16")
                    nc.scalar.activation(st["r16"][:], l16[:],
                                         AF.Exp, scale=-1.0)

                def s3():
                    rec1 = normp.tile([1, 2048], F32, name="rec1")
                    nc.sync.dma_start(rec1[:], st["r16"][:])
                    bcst = normp.tile([64, 2048], F32, name="bc")
                    nc.gpsimd.partition_broadcast(bcst[:], rec1[:])
                    for pi, attn in enumerate((attn01, attn23)):
                        for hh in range(2):
                            cs_ = 1024 * pi + 512 * hh
                            nc.vector.tensor_mul(
                                attn[64 * hh:64 * hh + 64, :],
                                pvc[0:64, cs_:cs_ + 512],
                                bcst[:, cs_:cs_ + 512])
                    nprog["done"] += 1
                return [s1, s2, s3]

            def attention(b, jp):
                qb = b * S + jp * 512
                nkt = 4 * jp + 4
                attn01 = attnp.tile([128, 512], BF16, name="at01")
                attn23 = attnp.tile([128, 512], BF16, name="at23")
                pvc = normp.tile([65, 2048], F32, name="pvc")
                for pi, (QRI, attn) in enumerate(
                        ((QRI_A, attn01), (QRI_B, attn23))):
                    PVs = pvps.tile([65, 1024], F32, name="PV")
                    pgq = deque()
                    for kt in range(nkt):
                        kc = b * S + kt * 128
                        r = kt - (nkt - 4)
                        cs0 = max(0, 128 * r)   # live q-col start
                        csl = slice(cs0, 512)
                        Sg = sps.tile([128, 1024], F32, name="Sg")
                        pg = probsp.tile([128, 1024], BF16, name="pg")
                        for hh in range(2):
                            hs = hh * 512
                            nc.tensor.matmul(
                                Sg[:, hs + cs0:hs + 512],
                                KRI2[64 * hh:64 * hh + 64, kc:kc + 128],
                                QRI[64 * hh:64 * hh + 64,
                                    qb + cs0:qb + 512],
                                start=True, stop=True,
                                tile_position=(64 * hh, 0))
                        sgv = Sg.rearrange(
                            "p (h c) -> p h c", h=2)[:, :, cs0:512]
                        pgv = pg.rearrange(
                            "p (h c) -> p h c", h=2)[:, :, cs0:512]
                        nc.scalar.activation(pgv, sgv, AF.Exp, scale=0.125)
                        if r >= 0:
                            pgt = pg.rearrange(
                                "p (h c) -> p h c",
                                h=2)[:, :, cs0:cs0 + 128]
                            nc.vector.tensor_mul(pgt, pgt, maskT3[:])
                        pgq.append((kt, pg, csl))
                        if len(pgq) > 2:
                            emit_pv(PVs, b, *pgq.popleft(), stop=False)
                        if pi == 0 and kt in (1, 2, 3) and norm_q:
                            norm_q.popleft()()
                        drain_wo(2 if len(pending) > 16 else 1)
                    while pgq:
                        item = pgq.popleft()
                        emit_pv(PVs, b, *item, stop=not pgq)
                    # free the PV banks asap; norm happens once per pair
                    nc.vector.tensor_copy(
                        pvc[:, 1024 * pi:1024 * pi + 1024], PVs[:])
                norm_q.extend(make_norm(pvc, attn01, attn23))
                enqueue_wo(attn01, attn23, qb)

            # ---------------- fused emission stream
            stream = [("p", 0), ("p", 1), ("a", 0, 0), ("p", 2),
                      ("a", 0, 1), ("p", 3), ("a", 0, 2), ("p", 4),
                      ("a", 0, 3), ("p", 5), ("a", 1, 0), ("p", 6),
                      ("a", 1, 1), ("p", 7), ("a", 1, 2), ("a", 1, 3)]
            next_xt = 2
            for step in stream:
                if step[0] == "p":
                    proj(step[1])
                else:
                    if next_xt < 8:
                        issue_xt(next_xt)
                        next_xt += 1
                    attention(step[1], step[2])
            while norm_q:
                norm_q.popleft()()
            drain_wo(1 << 30)

    nc.compile()
    bacc.get_activation_tables = _orig_gat
    return nc


def _prep_inputs(x, freqs_cos, freqs_sin, wq, wk, wv, wo):
    from ml_dtypes import bfloat16
    xf = np.asarray(x, np.float32).reshape(T, D)
    xTf = np.ascontiguousarray(xf.T).astype(bfloat16)      # [D, T]
    # tile-contiguous layout: tile (d, j) = xT[d*128:(d+1)*128, j*512:...]
    xTt = np.ascontiguousarray(
        xTf.reshape(16, 128, 8, 512).transpose(0, 2, 1, 3)
    ).reshape(16 * 8 * 128, 512)
    wq = np.asarray(wq, np.float32)
    wk = np.asarray(wk, np.float32)
    wv = np.asarray(wv, np.float32)
    wo = np.asarray(wo, np.float32)
    fc = np.asarray(freqs_cos, np.float32)
    fs = np.asarray(freqs_sin, np.float32)

    c4 = np.ascontiguousarray(np.tile(fc.T, (4, 1))).astype(bfloat16)
    s4 = np.ascontiguousarray(np.tile(fs.T, (4, 1))).astype(bfloat16)
    kt = np.arange(128)[:, None]
    qt = np.arange(128)[None, :]
    tri = (kt <= qt).astype(np.float32)
    maskT = np.ascontiguousarray(np.tile(tri, (1, 2))).astype(bfloat16)
    ev = np.arange(0, 64, 2)
    od = np.arange(1, 64, 2)

    in_maps = []
    for c in range(NCORES):
        qreal = np.concatenate([(4 * c + h) * 64 + ev for h in range(4)])
        qimag = np.concatenate([(4 * c + h) * 64 + od for h in range(4)])
        Wc = np.concatenate([wq[qreal], wq[qimag], wk[c * 64 + ev],
                             wk[c * 64 + od], wv[c * 64:(c + 1) * 64]], axis=0)
        in_maps.append({
            "xTt": xTt,
            "wqkvT": np.ascontiguousarray(Wc.T).astype(bfloat16),
            "woT": np.ascontiguousarray(
                wo[:, c * 256:(c + 1) * 256].T).astype(bfloat16),
            "c4": c4, "s4": s4, "maskT": maskT,
        })
    return in_maps


def _run(in_maps, trace=False, **kw):
    from concourse import bass_utils
    if "nc" not in _cache:
        _cache["nc"] = _build()
    return bass_utils.run_bass_kernel_spmd(
        _cache["nc"], in_maps, core_ids=list(range(NCORES)), trace=trace, **kw)


def kernel(x, freqs_cos, freqs_sin, wq, wk, wv, wo):
    in_maps = _prep_inputs(x, freqs_cos, freqs_sin, wq, wk, wv, wo)
    res = _run(in_maps)
    out = np.zeros((T, D), np.float64)
    for c in range(NCORES):
        out += np.asarray(res.results[c]["o"], np.float32)
    return out.astype(np.float32).reshape(B, S, D)


# revision 11
# speedup vs baseline: 1.2675x; 1.0105x over previous
"""Trainium2 Bass kernel for nn_Attention_14542759264705.

Dense transformer attention: QKV proj + interleaved RoPE + GQA causal
attention (32 q heads / 8 kv heads, hd=64) + output proj, fp32 in/out.

Sharding: tensor-parallel over kv-head groups across 8 cores. Core c owns
q heads 4c..4c+3 and kv head c; each core computes a partial output and
the host sums the 8 partials.

v3 (vs the v2 two-phase baseline):
  - Single fused PE stream: projection j-chunks are interleaved with
    attention pairs (proj0, proj1, A00, proj2, A01, ... A13) so the PE
    never parks at phase/pair boundaries (v2 lost ~146us to HAM
    re-throttling during starvation windows).
  - Warm-up matmuls at t=0 (no DMA deps) lift the PE clock gate before
    the first real matmul's data lands.
  - Dual-queue DMA: x tiles + weights alternate between the Sync and
    Scalar hardware-DGE queues (~2x ingest); x is tile-contiguous in
    DRAM so each [128,512] tile is one linear 128KB read.
  - Causal trim at 128-col granularity (v2 trimmed at 256) for scores,
    exp, and PV; tri-mask multiply is a single [128,2,128] DVE op.
  - Softmax denominators: [1,2048] row is DMA-reshaped to [16,128], so
    the Ln/Exp reciprocal costs ~0.7us of ACT instead of 4.6us.
  - wo psum->sbuf casts alternate DVE/ACT to split the ~100us cast load.
  - Projection channels run sequentially through one [128,512] psum slot
    pool (shared with wo + V-transposes) so proj+attention coexist in
    the 8 PSUM banks.
"""
import numpy as np

B, S, D = 2, 2048, 2048
T = B * S
NH, NKV, HD = 32, 8, 64
NCORES = 8

_cache = {}


def _build():
    from collections import deque
    from itertools import cycle

    import concourse.bacc as bacc
    import concourse.mybir as mybir
    import concourse.tile as tile
    from concourse.masks import make_identity

    F32 = mybir.dt.float32
    BF16 = mybir.dt.bfloat16
    AF = mybir.ActivationFunctionType

    # Force Exp/Ln/Copy onto the single combined act table set so the
    # compiler never inserts per-call ACT_TABLE_LOADs between exps and
    # the Ln/Exp reciprocal.
    from concourse.hw_specs import get_activation_tables as _gat

    def _patched_tables(arch):
        tabs = _gat(arch)
        key = "natural_log_exp_and_others"
        comb = tabs[key]
        return {n: (s if n == key else (s - comb)) for n, s in tabs.items()}

    _orig_gat = bacc.get_activation_tables
    bacc.get_activation_tables = _patched_tables

    nc = bacc.Bacc("TRN2", target_bir_lowering=False, debug=False,
                   num_devices=NCORES)
    # x tiles stored tile-contiguous: tile (d, j) at rows (d*8+j)*128
    xTt = nc.dram_tensor("xTt", [16 * 8 * 128, 512], BF16,
                         kind="ExternalInput").ap()
    wqkvT = nc.dram_tensor("wqkvT", [D, 384], BF16, kind="ExternalInput").ap()
    woT = nc.dram_tensor("woT", [256, D], BF16, kind="ExternalInput").ap()
    c4 = nc.dram_tensor("c4", [128, S], BF16, kind="ExternalInput").ap()
    s4 = nc.dram_tensor("s4", [128, S], BF16, kind="ExternalInput").ap()
    maskT = nc.dram_tensor("maskT", [128, 256], BF16,
                           kind="ExternalInput").ap()
    o = nc.dram_tensor("o", [T, D], BF16, kind="ExternalOutput").ap()

    with tile.TileContext(nc) as tc:
        with tc.tile_pool(name="res", bufs=1) as res, \
             tc.tile_pool(name="xtp", bufs=2) as xtp, \
             tc.tile_pool(name="ropet", bufs=2) as rp, \
             tc.tile_pool(name="probs", bufs=4) as probsp, \
             tc.tile_pool(name="attnp", bufs=2) as attnp, \
             tc.tile_pool(name="normp", bufs=2) as normp, \
             tc.tile_pool(name="outp", bufs=2) as outp, \
             tc.tile_pool(name="mmps", bufs=2, space="PSUM") as mmps, \
             tc.tile_pool(name="sps", bufs=2, space="PSUM") as sps, \
             tc.tile_pool(name="pvps", bufs=1, space="PSUM") as pvps:

            ident64 = res.tile([64, 64], BF16)
            make_identity(nc, ident64[:])
            c4_sb = res.tile([128, S], BF16)
            s4_sb = res.tile([128, S], BF16)
            maskT_sb = res.tile([128, 256], BF16)
            maskT3 = maskT_sb.rearrange("p (h c) -> p h c", c=128)

            QRI_A = res.tile([128, T], BF16)   # [h0r h0i h1r h1i] x tokens
            QRI_B = res.tile([128, T], BF16)   # [h2r h2i h3r h3i]
            KRI2 = res.tile([128, T], BF16)    # [Kr Ki Kr Ki]
            Vt_sb = res.tile([128, 32 * 65], BF16)  # kt-tile k at cols k*65
            Vt3 = Vt_sb.rearrange("p (k c) -> p k c", c=65)
            wqkv_r = res.tile([128, 16 * 384], BF16)
            woT_r = res.tile([128, 2 * D], BF16)
            ones32 = res.tile([128, 32], BF16)
            nc.gpsimd.memset(ones32[:], 1.0)
            nc.vector.tensor_copy(Vt3[:, :, 64], ones32[:])
            # ---- DMA issue: alternate hardware-DGE queues (Sync, Scalar)
            qcyc = cycle((nc.sync, nc.scalar))

            def dma(dst, src):
                next(qcyc).dma_start(dst, src)

            xt_tiles = {}

            def issue_xt(j):
                xts = []
                for d in range(16):
                    xt = xtp.tile([128, 512], BF16, name=f"xt{d}")
                    dma(xt[:], xTt[(d * 8 + j) * 128:(d * 8 + j + 1) * 128, :])
                    xts.append(xt)
                    if j == 0:
                        dma(wqkv_r[:, d * 384:(d + 1) * 384],
                            wqkvT[d * 128:(d + 1) * 128, :])
                xt_tiles[j] = xts
                if j == 0:
                    dma(c4_sb[:], c4[:])
                    dma(s4_sb[:], s4[:])
                elif j == 1:
                    dma(maskT_sb[:], maskT[:])
                    for t in range(2):
                        dma(woT_r[:, t * D:(t + 1) * D],
                            woT[t * 128:(t + 1) * 128, :])

            issue_xt(0)
            issue_xt(1)

            # ---------------- projection + rope for one 512-token chunk
            def proj(j):
                xts = xt_tiles.pop(j)
                kb = rp.tile([64, 512], BF16, name="kb")
                vsb = rp.tile([64, 512], BF16, name="vsb")
                qrb = rp.tile([128, 512], BF16, name="qrb")
                qib = rp.tile([128, 512], BF16, name="qib")
                for ch in range(3):
                    ps = mmps.tile([128, 512], F32, name="mm")
                    for d in range(16):
                        nc.tensor.matmul(
                            ps[:],
                            wqkv_r[:, d * 384 + ch * 128:
                                   d * 384 + (ch + 1) * 128],
                            xts[d][:], start=(d == 0), stop=(d == 15))
                    # stage before the next tile() call recycles the slot
                    if ch == 0:
                        nc.scalar.copy(qrb[:], ps[:])
                    elif ch == 1:
                        nc.scalar.copy(qib[:], ps[:])
                    else:
                        nc.scalar.copy(kb[:], ps[0:64, :])
                        nc.scalar.copy(vsb[:], ps[64:128, :])
                    drain_wo(1)
                tb = j * 512
                bc = (j % 4) * 512
                cs = c4_sb[:, bc:bc + 512]
                sn = s4_sb[:, bc:bc + 512]
                cs32 = c4_sb[0:32, bc:bc + 512]
                sn32 = s4_sb[0:32, bc:bc + 512]
                u1 = rp.tile([32, 512], BF16, name="u1", bufs=1)
                u2 = rp.tile([32, 512], BF16, name="u2", bufs=1)
                u3 = rp.tile([32, 512], BF16, name="u3", bufs=1)
                u4 = rp.tile([32, 512], BF16, name="u4", bufs=1)
                cs32b = c4_sb[32:64, bc:bc + 512]
                sn32b = s4_sb[32:64, bc:bc + 512]
                nc.vector.tensor_mul(u1[:], kb[0:32, :], cs32)
                nc.vector.tensor_mul(u2[:], kb[32:64, :], sn32b)
                nc.vector.tensor_mul(u3[:], kb[0:32, :], sn32)
                nc.vector.tensor_mul(u4[:], kb[32:64, :], cs32b)
                for g in (0, 64):
                    nc.vector.tensor_sub(
                        KRI2[g:g + 32, tb:tb + 512], u1[:], u2[:])
                for g in (32, 96):
                    nc.vector.tensor_add(
                        KRI2[g:g + 32, tb:tb + 512], u3[:], u4[:])
                t1 = rp.tile([128, 512], BF16, name="t1", bufs=1)
                t2 = rp.tile([128, 512], BF16, name="t2", bufs=1)
                t3 = rp.tile([128, 512], BF16, name="t3", bufs=1)
                t4 = rp.tile([128, 512], BF16, name="t4", bufs=1)
                nc.vector.tensor_mul(t1[:], qrb[:], cs)
                nc.vector.tensor_mul(t3[:], qrb[:], sn)
                nc.vector.tensor_mul(t2[:], qib[:], sn)
                nc.vector.tensor_mul(t4[:], qib[:], cs)
                for hh in range(4):
                    dst = QRI_A if hh < 2 else QRI_B
                    base = (hh % 2) * 64
                    nc.vector.tensor_sub(
                        dst[base:base + 32, tb:tb + 512],
                        t1[32 * hh:32 * hh + 32, :],
                        t2[32 * hh:32 * hh + 32, :])
                    nc.vector.tensor_add(
                        dst[base + 32:base + 64, tb:tb + 512],
                        t3[32 * hh:32 * hh + 32, :],
                        t4[32 * hh:32 * hh + 32, :])
                # V transpose tiles (PE, psum slot shared with proj/wo)
                vtp = mmps.tile([128, 256], BF16, name="vtp", tag="mm")
                for i in range(4):
                    nc.tensor.transpose(
                        vtp[:, i * 64:(i + 1) * 64],
                        vsb[:, i * 128:(i + 1) * 128], ident64[:])
                vtp3 = vtp.rearrange("p (k c) -> p k c", c=64)
                nc.vector.tensor_copy(
                    Vt3[:, j * 4:j * 4 + 4, 0:64], vtp3[:])
                drain_wo(1)

            # ---------------- attention machinery
            pending = deque()
            norm_q = deque()
            nprog = {"enq": 0, "pi_done": 0}
            NPAIRS = 8

            def enqueue_wo(attn01, attn23, qb):
                state = {}
                pid = nprog["enq"]
                nprog["enq"] += 1
                for qs in range(4):
                    for do in range(4):
                        pending.append(
                            (pid, attn01, attn23, qb, qs, do, state))

            def drain_wo(n):
                for _ in range(n):
                    if not pending:
                        return
                    if pending[0][0] >= nprog["pi_done"] // 2:
                        return   # this pair's attn not normed yet
                    pid, attn01, attn23, qb, qs, do, state = \
                        pending.popleft()
                    last = pid == NPAIRS - 1
                    if do == 0:
                        state[qs] = outp.tile([128, D], BF16, name="osb")
                    osb = state[qs]
                    Ops = mmps.tile([128, 512], F32, name="mm")
                    nc.tensor.matmul(
                        Ops[:], attn01[:, qs * 128:(qs + 1) * 128],
                        woT_r[:, do * 512:(do + 1) * 512],
                        start=True, stop=False)
                    nc.tensor.matmul(
                        Ops[:], attn23[:, qs * 128:(qs + 1) * 128],
                        woT_r[:, D + do * 512:D + (do + 1) * 512],
                        start=False, stop=True)
                    if last and do % 2 == 1:
                        nc.scalar.copy(
                            osb[:, do * 512:(do + 1) * 512], Ops[:])
                    else:
                        nc.vector.tensor_copy(
                            osb[:, do * 512:(do + 1) * 512], Ops[:])
                    if do == 3:
                        qq = qb + qs * 128
                        eng = nc.scalar if (last and qs % 2) else nc.sync
                        eng.dma_start(o[qq:qq + 128, :], osb[:])

            def emit_pv(PVs, b, pkt, ppg, pcsl, stop):
                vt = Vt3[:, b * 16 + pkt, :]
                st = (pkt == 0)
                for hh in range(2):
                    hs = hh * 512
                    nc.tensor.matmul(
                        PVs[:, hs + pcsl.start:hs + pcsl.stop], vt,
                        ppg[:, hs + pcsl.start:hs + pcsl.stop],
                        start=st, stop=stop)

            def make_norm(pvc, pi, attn):
                st = {}
                cb = 1024 * pi

                def s1():
                    st["d8"] = normp.tile([8, 128], F32, name="d8")
                    nc.sync.dma_start(st["d8"][:],
                                      pvc[64:65, cb:cb + 1024])

                def s2():
                    l8 = normp.tile([8, 128], F32, name="l8")
                    nc.scalar.activation(l8[:], st["d8"][:], AF.Ln)
                    st["r8"] = normp.tile([8, 128], F32, name="r8")
                    nc.scalar.activation(st["r8"][:], l8[:],
                                         AF.Exp, scale=-1.0)

                def s3():
                    rec1 = normp.tile([1, 1024], F32, name="rec1")
                    nc.sync.dma_start(rec1[:], st["r8"][:])
                    bcst = normp.tile([64, 1024], F32, name="bc")
                    nc.gpsimd.partition_broadcast(bcst[:], rec1[:])
                    for hh in range(2):
                        nc.vector.tensor_mul(
                            attn[64 * hh:64 * hh + 64, :],
                            pvc[0:64, cb + 512 * hh:cb + 512 * hh + 512],
                            bcst[:, 512 * hh:512 * hh + 512])
                    nprog["pi_done"] += 1
                return [s1, s2, s3]

            def attention(b, jp):
                qb = b * S + jp * 512
                nkt = 4 * jp + 4
                attn01 = attnp.tile([128, 512], BF16, name="at01")
                attn23 = attnp.tile([128, 512], BF16, name="at23")
                pvc = normp.tile([65, 2048], F32, name="pvc")
                for pi, (QRI, attn) in enumerate(
                        ((QRI_A, attn01), (QRI_B, attn23))):
                    PVs = pvps.tile([65, 1024], F32, name="PV")
                    pgq = deque()
                    for kt in range(nkt):
                        kc = b * S + kt * 128
                        r = kt - (nkt - 4)
                        cs0 = max(0, 128 * r)   # live q-col start
                        csl = slice(cs0, 512)
                        Sg = sps.tile([128, 1024], F32, name="Sg")
                        pg = probsp.tile([128, 1024], BF16, name="pg")
                        for hh in range(2):
                            hs = hh * 512
                            nc.tensor.matmul(
                                Sg[:, hs + cs0:hs + 512],
                                KRI2[64 * hh:64 * hh + 64, kc:kc + 128],
                                QRI[64 * hh:64 * hh + 64,
                                    qb + cs0:qb + 512],
                                start=True, stop=True,
                                tile_position=(64 * hh, 0))
                        sgv = Sg.rearrange(
                            "p (h c) -> p h c", h=2)[:, :, cs0:512]
                        pgv = pg.rearrange(
                            "p (h c) -> p h c", h=2)[:, :, cs0:512]
                        nc.scalar.activation(pgv, sgv, AF.Exp, scale=0.125)
                        if r >= 0:
                            pgt = pg.rearrange(
                                "p (h c) -> p h c",
                                h=2)[:, :, cs0:cs0 + 128]
                            nc.vector.tensor_mul(pgt, pgt, maskT3[:])
                        pgq.append((kt, pg, csl))
                        if len(pgq) > 2:
                            emit_pv(PVs, b, *pgq.popleft(), stop=False)
                        if kt in (1, 2, 3) and norm_q:
                            norm_q.popleft()()
                        drain_wo(2 if len(pending) > 8 else 1)
                    while pgq:
                        item = pgq.popleft()
                        emit_pv(PVs, b, *item, stop=not pgq)
                    # free the PV banks asap; norm (per pi) is deferred
                    # into the following kt loops
                    nc.vector.tensor_copy(
                        pvc[:, 1024 * pi:1024 * pi + 1024], PVs[:])
                    norm_q.extend(make_norm(pvc, pi, attn))
                enqueue_wo(attn01, attn23, qb)

            # ---------------- fused emission stream
            stream = [("p", 0), ("p", 1), ("a", 0, 0), ("p", 2),
                      ("a", 0, 1), ("p", 3), ("a", 0, 2), ("p", 4),
                      ("a", 0, 3), ("p", 5), ("a", 1, 0), ("p", 6),
                      ("a", 1, 1), ("p", 7), ("a", 1, 2), ("a", 1, 3)]
            next_xt = 2
            for step in stream:
                if step[0] == "p":
                    proj(step[1])
                else:
                    if next_xt < 8:
                        issue_xt(next_xt)
                        next_xt += 1
                    attention(step[1], step[2])
            while norm_q:
                norm_q.popleft()()
            drain_wo(1 << 30)

    nc.compile()
    bacc.get_activation_tables = _orig_gat
    return nc


def _prep_inputs(x, freqs_cos, freqs_sin, wq, wk, wv, wo):
    from ml_dtypes import bfloat16
    xf = np.asarray(x, np.float32).reshape(T, D)
    xTf = np.ascontiguousarray(xf.T).astype(bfloat16)      # [D, T]
    # tile-contiguous layout: tile (d, j) = xT[d*128:(d+1)*128, j*512:...]
    xTt = np.ascontiguousarray(
        xTf.reshape(16, 128, 8, 512).transpose(0, 2, 1, 3)
    ).reshape(16 * 8 * 128, 512)
    wq = np.asarray(wq, np.float32)
    wk = np.asarray(wk, np.float32)
    wv = np.asarray(wv, np.float32)
    wo = np.asarray(wo, np.float32)
    fc = np.asarray(freqs_cos, np.float32)
    fs = np.asarray(freqs_sin, np.float32)

    c4 = np.ascontiguousarray(np.tile(fc.T, (4, 1))).astype(bfloat16)
    s4 = np.ascontiguousarray(np.tile(fs.T, (4, 1))).astype(bfloat16)
    kt = np.arange(128)[:, None]
    qt = np.arange(128)[None, :]
    tri = (kt <= qt).astype(np.float32)
    maskT = np.ascontiguousarray(np.tile(tri, (1, 2))).astype(bfloat16)
    ev = np.arange(0, 64, 2)
    od = np.arange(1, 64, 2)

    in_maps = []
    for c in range(NCORES):
        qreal = np.concatenate([(4 * c + h) * 64 + ev for h in range(4)])
        qimag = np.concatenate([(4 * c + h) * 64 + od for h in range(4)])
        Wc = np.concatenate([wq[qreal], wq[qimag], wk[c * 64 + ev],
                             wk[c * 64 + od], wv[c * 64:(c + 1) * 64]], axis=0)
        in_maps.append({
            "xTt": xTt,
            "wqkvT": np.ascontiguousarray(Wc.T).astype(bfloat16),
            "woT": np.ascontiguousarray(
                wo[:, c * 256:(c + 1) * 256].T).astype(bfloat16),
            "c4": c4, "s4": s4, "maskT": maskT,
        })
    return in_maps


def _run(in_maps, trace=False, **kw):
    from concourse import bass_utils
    if "nc" not in _cache:
        _cache["nc"] = _build()
    return bass_utils.run_bass_kernel_spmd(
        _cache["nc"], in_maps, core_ids=list(range(NCORES)), trace=trace, **kw)


def kernel(x, freqs_cos, freqs_sin, wq, wk, wv, wo):
    in_maps = _prep_inputs(x, freqs_cos, freqs_sin, wq, wk, wv, wo)
    res = _run(in_maps)
    out = np.zeros((T, D), np.float64)
    for c in range(NCORES):
        out += np.asarray(res.results[c]["o"], np.float32)
    return out.astype(np.float32).reshape(B, S, D)


# revision 21
# speedup vs baseline: 1.3053x; 1.0298x over previous
"""Trainium2 Bass kernel for nn_Attention_14542759264705.

Dense transformer attention: QKV proj + interleaved RoPE + GQA causal
attention (32 q heads / 8 kv heads, hd=64) + output proj, fp32 in/out.

Sharding: tensor-parallel over kv-head groups across 8 cores. Core c owns
q heads 4c..4c+3 and kv head c; each core computes a partial output and
the host sums the 8 partials.

v3 (vs the v2 two-phase baseline):
  - Single fused PE stream: projection j-chunks are interleaved with
    attention pairs (proj0, proj1, A00, proj2, A01, ... A13) so the PE
    never parks at phase/pair boundaries (v2 lost ~146us to HAM
    re-throttling during starvation windows).
  - Warm-up matmuls at t=0 (no DMA deps) lift the PE clock gate before
    the first real matmul's data lands.
  - Dual-queue DMA: x tiles + weights alternate between the Sync and
    Scalar hardware-DGE queues (~2x ingest); x is tile-contiguous in
    DRAM so each [128,512] tile is one linear 128KB read.
  - Causal trim at 128-col granularity (v2 trimmed at 256) for scores,
    exp, and PV; tri-mask multiply is a single [128,2,128] DVE op.
  - Softmax denominators: [1,2048] row is DMA-reshaped to [16,128], so
    the Ln/Exp reciprocal costs ~0.7us of ACT instead of 4.6us.
  - wo psum->sbuf casts alternate DVE/ACT to split the ~100us cast load.
  - Projection channels run sequentially through one [128,512] psum slot
    pool (shared with wo + V-transposes) so proj+attention coexist in
    the 8 PSUM banks.
"""
import numpy as np

B, S, D = 2, 2048, 2048
T = B * S
NH, NKV, HD = 32, 8, 64
NCORES = 8

_cache = {}


def _build():
    from collections import deque
    from itertools import cycle

    import concourse.bacc as bacc
    import concourse.mybir as mybir
    import concourse.tile as tile
    from concourse.masks import make_identity

    F32 = mybir.dt.float32
    BF16 = mybir.dt.bfloat16
    AF = mybir.ActivationFunctionType

    # Force Exp/Ln/Copy onto the single combined act table set so the
    # compiler never inserts per-call ACT_TABLE_LOADs between exps and
    # the Ln/Exp reciprocal.
    from concourse.hw_specs import get_activation_tables as _gat

    def _patched_tables(arch):
        tabs = _gat(arch)
        key = "natural_log_exp_and_others"
        comb = tabs[key]
        return {n: (s if n == key else (s - comb)) for n, s in tabs.items()}

    _orig_gat = bacc.get_activation_tables
    bacc.get_activation_tables = _patched_tables

    nc = bacc.Bacc("TRN2", target_bir_lowering=False, debug=False,
                   num_devices=NCORES)
    # x stored j-major, partition-row major: xTt[j*128+p, d*512+c] holds
    # x^T element (d*128+p, j*512+c) -> each j-chunk is one plain 2D DMA
    xTt = nc.dram_tensor("xTt", [8 * 128, 16 * 512], BF16,
                         kind="ExternalInput").ap()
    wqkvT = nc.dram_tensor("wqkvT", [D, 384], BF16, kind="ExternalInput").ap()
    woT = nc.dram_tensor("woT", [256, D], BF16, kind="ExternalInput").ap()
    c4 = nc.dram_tensor("c4", [128, S], BF16, kind="ExternalInput").ap()
    s4 = nc.dram_tensor("s4", [128, S], BF16, kind="ExternalInput").ap()
    maskT = nc.dram_tensor("maskT", [128, 256], BF16,
                           kind="ExternalInput").ap()
    o = nc.dram_tensor("o", [T, D], BF16, kind="ExternalOutput").ap()

    with tile.TileContext(nc) as tc:
        with tc.tile_pool(name="res", bufs=1) as res, \
             tc.tile_pool(name="xtp", bufs=2) as xtp, \
             tc.tile_pool(name="ropet", bufs=2) as rp, \
             tc.tile_pool(name="probs", bufs=4) as probsp, \
             tc.tile_pool(name="attnp", bufs=2) as attnp, \
             tc.tile_pool(name="normp", bufs=2) as normp, \
             tc.tile_pool(name="outp", bufs=2) as outp, \
             tc.tile_pool(name="mmps", bufs=2, space="PSUM") as mmps, \
             tc.tile_pool(name="sps", bufs=2, space="PSUM") as sps, \
             tc.tile_pool(name="pvps", bufs=1, space="PSUM") as pvps:

            ident64 = res.tile([64, 64], BF16)
            make_identity(nc, ident64[:])
            c4_sb = res.tile([128, S], BF16)
            s4_sb = res.tile([128, S], BF16)
            maskT_sb = res.tile([128, 256], BF16)
            maskT3 = maskT_sb.rearrange("p (h c) -> p h c", c=128)

            QRI_A = res.tile([128, T], BF16)   # [h0r h0i h1r h1i] x tokens
            QRI_B = res.tile([128, T], BF16)   # [h2r h2i h3r h3i]
            KRI2 = res.tile([128, T], BF16)    # [Kr Ki Kr Ki]
            Vt_sb = res.tile([128, 32 * 65], BF16)  # kt-tile k at cols k*65
            Vt3 = Vt_sb.rearrange("p (k c) -> p k c", c=65)
            wqkv_r = res.tile([128, 16 * 384], BF16)
            woT_r = res.tile([128, 2 * D], BF16)
            ones32 = res.tile([128, 32], BF16)
            nc.gpsimd.memset(ones32[:], 1.0)
            nc.vector.tensor_copy(Vt3[:, :, 64], ones32[:])
            # ---- DMA plan: Sync queue carries x tiles + o writes + norm
            # reshapes; Scalar queue carries only the weight preload (so
            # phase-2 exps are never stuck behind DMA blocks).
            xt_tiles = {}

            def issue_xt(j):
                xall = xtp.tile([128, 16 * 512], BF16, name="xall")
                if j < 2:
                    # per-tile loads so proj(j)'s d-loop starts asap
                    for d in range(16):
                        nc.sync.dma_start(
                            xall[:, d * 512:(d + 1) * 512],
                            xTt[j * 128:(j + 1) * 128,
                                d * 512:(d + 1) * 512])
                else:
                    # prefetched a pair ahead; one 2MB linear DMA
                    nc.sync.dma_start(
                        xall[:], xTt[j * 128:(j + 1) * 128, :])
                xt_tiles[j] = xall
                if j == 0:
                    for d in range(16):
                        nc.scalar.dma_start(
                            wqkv_r[:, d * 384:(d + 1) * 384],
                            wqkvT[d * 128:(d + 1) * 128, :])
                    nc.scalar.dma_start(c4_sb[:], c4[:])
                    nc.scalar.dma_start(s4_sb[:], s4[:])
                elif j == 1:
                    nc.scalar.dma_start(maskT_sb[:], maskT[:])
                    for t in range(2):
                        nc.scalar.dma_start(woT_r[:, t * D:(t + 1) * D],
                                            woT[t * 128:(t + 1) * 128, :])

            issue_xt(0)

            # ---------------- projection + rope for one 512-token chunk
            def proj(j):
                xall = xt_tiles.pop(j)
                kb = rp.tile([64, 512], BF16, name="kb")
                vsb = rp.tile([64, 512], BF16, name="vsb")
                qrb = rp.tile([128, 512], BF16, name="qrb")
                qib = rp.tile([128, 512], BF16, name="qib")
                for ch in range(3):
                    ps = mmps.tile([128, 512], F32, name="mm")
                    for d in range(16):
                        nc.tensor.matmul(
                            ps[:],
                            wqkv_r[:, d * 384 + ch * 128:
                                   d * 384 + (ch + 1) * 128],
                            xall[:, d * 512:(d + 1) * 512],
                            start=(d == 0), stop=(d == 15))
                    # stage before the next tile() call recycles the slot
                    if ch == 0:
                        nc.scalar.copy(qrb[:], ps[:])
                    elif ch == 1:
                        nc.scalar.copy(qib[:], ps[:])
                    else:
                        nc.scalar.copy(kb[:], ps[0:64, :])
                        nc.scalar.copy(vsb[:], ps[64:128, :])
                    drain_wo(1)
                tb = j * 512
                bc = (j % 4) * 512
                cs = c4_sb[:, bc:bc + 512]
                sn = s4_sb[:, bc:bc + 512]
                cs32 = c4_sb[0:32, bc:bc + 512]
                sn32 = s4_sb[0:32, bc:bc + 512]
                u1 = rp.tile([32, 512], BF16, name="u1", bufs=1)
                u2 = rp.tile([32, 512], BF16, name="u2", bufs=1)
                u3 = rp.tile([32, 512], BF16, name="u3", bufs=1)
                u4 = rp.tile([32, 512], BF16, name="u4", bufs=1)
                cs32b = c4_sb[32:64, bc:bc + 512]
                sn32b = s4_sb[32:64, bc:bc + 512]
                nc.vector.tensor_mul(u1[:], kb[0:32, :], cs32)
                nc.vector.tensor_mul(u2[:], kb[32:64, :], sn32b)
                nc.vector.tensor_mul(u3[:], kb[0:32, :], sn32)
                nc.vector.tensor_mul(u4[:], kb[32:64, :], cs32b)
                for g in (0, 64):
                    nc.vector.tensor_sub(
                        KRI2[g:g + 32, tb:tb + 512], u1[:], u2[:])
                for g in (32, 96):
                    nc.vector.tensor_add(
                        KRI2[g:g + 32, tb:tb + 512], u3[:], u4[:])
                t1 = rp.tile([128, 512], BF16, name="t1", bufs=1)
                t2 = rp.tile([128, 512], BF16, name="t2", bufs=1)
                t3 = rp.tile([128, 512], BF16, name="t3", bufs=1)
                t4 = rp.tile([128, 512], BF16, name="t4", bufs=1)
                nc.vector.tensor_mul(t1[:], qrb[:], cs)
                nc.vector.tensor_mul(t3[:], qrb[:], sn)
                nc.vector.tensor_mul(t2[:], qib[:], sn)
                nc.vector.tensor_mul(t4[:], qib[:], cs)
                for hh in range(4):
                    dst = QRI_A if hh < 2 else QRI_B
                    base = (hh % 2) * 64
                    nc.vector.tensor_sub(
                        dst[base:base + 32, tb:tb + 512],
                        t1[32 * hh:32 * hh + 32, :],
                        t2[32 * hh:32 * hh + 32, :])
                    nc.vector.tensor_add(
                        dst[base + 32:base + 64, tb:tb + 512],
                        t3[32 * hh:32 * hh + 32, :],
                        t4[32 * hh:32 * hh + 32, :])
                # V transpose tiles (PE, psum slot shared with proj/wo)
                vtp = mmps.tile([128, 256], BF16, name="vtp", tag="mm")
                for i in range(4):
                    nc.tensor.transpose(
                        vtp[:, i * 64:(i + 1) * 64],
                        vsb[:, i * 128:(i + 1) * 128], ident64[:])
                vtp3 = vtp.rearrange("p (k c) -> p k c", c=64)
                nc.vector.tensor_copy(
                    Vt3[:, j * 4:j * 4 + 4, 0:64], vtp3[:])
                drain_wo(1)

            # ---------------- attention machinery
            pending = deque()
            norm_q = deque()
            nprog = {"enq": 0, "pi_done": 0}
            NPAIRS = 8

            def enqueue_wo(attn01, attn23, qb):
                state = {}
                pid = nprog["enq"]
                nprog["enq"] += 1
                for qs in range(4):
                    for do in range(4):
                        pending.append(
                            (pid, attn01, attn23, qb, qs, do, state))

            def drain_wo(n, reserve=0):
                for _ in range(n):
                    if len(pending) <= reserve:
                        return
                    if pending[0][0] >= nprog["pi_done"] // 2:
                        return   # this pair's attn not normed yet
                    pid, attn01, attn23, qb, qs, do, state = \
                        pending.popleft()
                    last = pid == NPAIRS - 1
                    if do == 0:
                        state[qs] = outp.tile([128, D], BF16, name="osb")
                    osb = state[qs]
                    Ops = mmps.tile([128, 512], F32, name="mm")
                    nc.tensor.matmul(
                        Ops[:], attn01[:, qs * 128:(qs + 1) * 128],
                        woT_r[:, do * 512:(do + 1) * 512],
                        start=True, stop=False)
                    nc.tensor.matmul(
                        Ops[:], attn23[:, qs * 128:(qs + 1) * 128],
                        woT_r[:, D + do * 512:D + (do + 1) * 512],
                        start=False, stop=True)
                    if last and do % 2 == 1:
                        nc.scalar.copy(
                            osb[:, do * 512:(do + 1) * 512], Ops[:])
                    else:
                        nc.vector.tensor_copy(
                            osb[:, do * 512:(do + 1) * 512], Ops[:])
                    if do == 3:
                        qq = qb + qs * 128
                        eng = nc.scalar if (last and qs % 2) else nc.sync
                        eng.dma_start(o[qq:qq + 128, :], osb[:])

            def emit_pv(PVs, b, pkt, ppg, pcsl, stop):
                vt = Vt3[:, b * 16 + pkt, :]
                st = (pkt == 0)
                for hh in range(2):
                    hs = hh * 512
                    nc.tensor.matmul(
                        PVs[:, hs + pcsl.start:hs + pcsl.stop], vt,
                        ppg[:, hs + pcsl.start:hs + pcsl.stop],
                        start=st, stop=stop)

            def make_norm(pvc, pi, attn):
                st = {}
                cb = 1024 * pi

                def s1():
                    st["d8"] = normp.tile([8, 128], F32, name="d8")
                    nc.sync.dma_start(st["d8"][:],
                                      pvc[64:65, cb:cb + 1024])

                def s2():
                    l8 = normp.tile([8, 128], F32, name="l8")
                    nc.scalar.activation(l8[:], st["d8"][:], AF.Ln)
                    st["r8"] = normp.tile([8, 128], F32, name="r8")
                    nc.scalar.activation(st["r8"][:], l8[:],
                                         AF.Exp, scale=-1.0)

                def s3():
                    rec1 = normp.tile([1, 1024], F32, name="rec1")
                    nc.sync.dma_start(rec1[:], st["r8"][:])
                    bcst = normp.tile([64, 1024], F32, name="bc")
                    nc.gpsimd.partition_broadcast(bcst[:], rec1[:])
                    for hh in range(2):
                        nc.vector.tensor_mul(
                            attn[64 * hh:64 * hh + 64, :],
                            pvc[0:64, cb + 512 * hh:cb + 512 * hh + 512],
                            bcst[:, 512 * hh:512 * hh + 512])
                    nprog["pi_done"] += 1
                return [s1, s2, s3]

            def attention(b, jp):
                qb = b * S + jp * 512
                nkt = 4 * jp + 4
                # during the final pair, hold back a few normed wo steps so
                # the tail's norm-latency window has PE work queued
                rsv = 6 if (b, jp) == (1, 3) else 0
                attn01 = attnp.tile([128, 512], BF16, name="at01")
                attn23 = attnp.tile([128, 512], BF16, name="at23")
                pvc = normp.tile([65, 2048], F32, name="pvc")
                for pi, (QRI, attn) in enumerate(
                        ((QRI_A, attn01), (QRI_B, attn23))):
                    drain_wo(2, rsv)
                    PVs = pvps.tile([65, 1024], F32, name="PV")
                    pgq = deque()
                    for kt in range(nkt):
                        kc = b * S + kt * 128
                        r = kt - (nkt - 4)
                        cs0 = max(0, 128 * r)   # live q-col start
                        csl = slice(cs0, 512)
                        Sg = sps.tile([128, 1024], F32, name="Sg")
                        pg = probsp.tile([128, 1024], BF16, name="pg")
                        for hh in range(2):
                            hs = hh * 512
                            nc.tensor.matmul(
                                Sg[:, hs + cs0:hs + 512],
                                KRI2[64 * hh:64 * hh + 64, kc:kc + 128],
                                QRI[64 * hh:64 * hh + 64,
                                    qb + cs0:qb + 512],
                                start=True, stop=True,
                                tile_position=(64 * hh, 0))
                        sgv = Sg.rearrange(
                            "p (h c) -> p h c", h=2)[:, :, cs0:512]
                        pgv = pg.rearrange(
                            "p (h c) -> p h c", h=2)[:, :, cs0:512]
                        nc.scalar.activation(pgv, sgv, AF.Exp, scale=0.125)
                        if r >= 0:
                            pgt = pg.rearrange(
                                "p (h c) -> p h c",
                                h=2)[:, :, cs0:cs0 + 128]
                            nc.vector.tensor_mul(pgt, pgt, maskT3[:])
                        pgq.append((kt, pg, csl))
                        if len(pgq) > 2:
                            emit_pv(PVs, b, *pgq.popleft(), stop=False)
                        if kt in (1, 2, 3) and norm_q:
                            norm_q.popleft()()
                        drain_wo(2 if len(pending) > 8 else 1, rsv)
                    while pgq:
                        item = pgq.popleft()
                        emit_pv(PVs, b, *item, stop=not pgq)
                    drain_wo(2, rsv)
                    # free the PV banks asap; norm (per pi) is deferred
                    # into the following kt loops
                    nc.vector.tensor_copy(
                        pvc[:, 1024 * pi:1024 * pi + 1024], PVs[:])
                    norm_q.extend(make_norm(pvc, pi, attn))
                enqueue_wo(attn01, attn23, qb)

            # ---------------- fused emission stream
            stream = [("p", 0), ("p", 1), ("a", 0, 0), ("p", 2),
                      ("a", 0, 1), ("p", 3), ("a", 0, 2), ("p", 4),
                      ("a", 0, 3), ("p", 5), ("a", 1, 0), ("p", 6),
                      ("a", 1, 1), ("p", 7), ("a", 1, 2), ("a", 1, 3)]
            next_xt = 2
            for step in stream:
                if step[0] == "p":
                    proj(step[1])
                    if step[1] == 0:
                        issue_xt(1)
                else:
                    if next_xt < 8:
                        issue_xt(next_xt)
                        next_xt += 1
                    attention(step[1], step[2])
            while norm_q:
                norm_q.popleft()()
            drain_wo(1 << 30)

    nc.compile()
    bacc.get_activation_tables = _orig_gat
    return nc


def _prep_inputs(x, freqs_cos, freqs_sin, wq, wk, wv, wo):
    from ml_dtypes import bfloat16
    xf = np.asarray(x, np.float32).reshape(T, D)
    xTf = np.ascontiguousarray(xf.T).astype(bfloat16)      # [D, T]
    # j-major p-row-major layout: xTt[j, p, d, c] = xT[d*128+p, j*512+c]
    xTt = np.ascontiguousarray(
        xTf.reshape(16, 128, 8, 512).transpose(2, 1, 0, 3)
    ).reshape(8 * 128, 16 * 512)
    wq = np.asarray(wq, np.float32)
    wk = np.asarray(wk, np.float32)
    wv = np.asarray(wv, np.float32)
    wo = np.asarray(wo, np.float32)
    fc = np.asarray(freqs_cos, np.float32)
    fs = np.asarray(freqs_sin, np.float32)

    c4 = np.ascontiguousarray(np.tile(fc.T, (4, 1))).astype(bfloat16)
    s4 = np.ascontiguousarray(np.tile(fs.T, (4, 1))).astype(bfloat16)
    kt = np.arange(128)[:, None]
    qt = np.arange(128)[None, :]
    tri = (kt <= qt).astype(np.float32)
    maskT = np.ascontiguousarray(np.tile(tri, (1, 2))).astype(bfloat16)
    ev = np.arange(0, 64, 2)
    od = np.arange(1, 64, 2)

    in_maps = []
    for c in range(NCORES):
        qreal = np.concatenate([(4 * c + h) * 64 + ev for h in range(4)])
        qimag = np.concatenate([(4 * c + h) * 64 + od for h in range(4)])
        Wc = np.concatenate([wq[qreal], wq[qimag], wk[c * 64 + ev],
                             wk[c * 64 + od], wv[c * 64:(c + 1) * 64]], axis=0)
        in_maps.append({
            "xTt": xTt,
            "wqkvT": np.ascontiguousarray(Wc.T).astype(bfloat16),
            "woT": np.ascontiguousarray(
                wo[:, c * 256:(c + 1) * 256].T).astype(bfloat16),
            "c4": c4, "s4": s4, "maskT": maskT,
        })
    return in_maps


def _run(in_maps, trace=False, **kw):
    from concourse import bass_utils
    if "nc" not in _cache:
        _cache["nc"] = _build()
    return bass_utils.run_bass_kernel_spmd(
        _cache["nc"], in_maps, core_ids=list(range(NCORES)), trace=trace, **kw)


def kernel(x, freqs_cos, freqs_sin, wq, wk, wv, wo):
    in_maps = _prep_inputs(x, freqs_cos, freqs_sin, wq, wk, wv, wo)
    res = _run(in_maps)
    out = np.zeros((T, D), np.float64)
    for c in range(NCORES):
        out += np.asarray(res.results[c]["o"], np.float32)
    return out.astype(np.float32).reshape(B, S, D)


# revision 28
# speedup vs baseline: 1.3168x; 1.0088x over previous
"""Trainium2 Bass kernel for nn_Attention_14542759264705.

Dense transformer attention: QKV proj + interleaved RoPE + GQA causal
attention (32 q heads / 8 kv heads, hd=64) + output proj, fp32 in/out.

Sharding: tensor-parallel over kv-head groups across 8 cores. Core c owns
q heads 4c..4c+3 and kv head c; each core computes a partial output and
the host sums the 8 partials.

v3 (vs the v2 two-phase baseline):
  - Single fused PE stream: projection j-chunks are interleaved with
    attention pairs (proj0, proj1, A00, proj2, A01, ... A13) so the PE
    never parks at phase/pair boundaries (v2 lost ~146us to HAM
    re-throttling during starvation windows).
  - Warm-up matmuls at t=0 (no DMA deps) lift the PE clock gate before
    the first real matmul's data lands.
  - Dual-queue DMA: x tiles + weights alternate between the Sync and
    Scalar hardware-DGE queues (~2x ingest); x is tile-contiguous in
    DRAM so each [128,512] tile is one linear 128KB read.
  - Causal trim at 128-col granularity (v2 trimmed at 256) for scores,
    exp, and PV; tri-mask multiply is a single [128,2,128] DVE op.
  - Softmax denominators: [1,2048] row is DMA-reshaped to [16,128], so
    the Ln/Exp reciprocal costs ~0.7us of ACT instead of 4.6us.
  - wo psum->sbuf casts alternate DVE/ACT to split the ~100us cast load.
  - Projection channels run sequentially through one [128,512] psum slot
    pool (shared with wo + V-transposes) so proj+attention coexist in
    the 8 PSUM banks.
"""
import numpy as np

B, S, D = 2, 2048, 2048
T = B * S
NH, NKV, HD = 32, 8, 64
NCORES = 8

_cache = {}


def _build():
    from collections import deque
    from itertools import cycle

    import concourse.bacc as bacc
    import concourse.mybir as mybir
    import concourse.tile as tile
    from concourse.masks import make_identity

    F32 = mybir.dt.float32
    BF16 = mybir.dt.bfloat16
    AF = mybir.ActivationFunctionType

    # Force Exp/Ln/Copy onto the single combined act table set so the
    # compiler never inserts per-call ACT_TABLE_LOADs between exps and
    # the Ln/Exp reciprocal.
    from concourse.hw_specs import get_activation_tables as _gat

    def _patched_tables(arch):
        tabs = _gat(arch)
        key = "natural_log_exp_and_others"
        comb = tabs[key]
        return {n: (s if n == key else (s - comb)) for n, s in tabs.items()}

    _orig_gat = bacc.get_activation_tables
    bacc.get_activation_tables = _patched_tables

    nc = bacc.Bacc("TRN2", target_bir_lowering=False, debug=False,
                   num_devices=NCORES)
    # x stored j-major, partition-row major: xTt[j*128+p, d*512+c] holds
    # x^T element (d*128+p, j*512+c) -> each j-chunk is one plain 2D DMA
    xTt = nc.dram_tensor("xTt", [8 * 128, 16 * 512], BF16,
                         kind="ExternalInput").ap()
    wqkvT = nc.dram_tensor("wqkvT", [D, 384], BF16, kind="ExternalInput").ap()
    woT = nc.dram_tensor("woT", [256, D], BF16, kind="ExternalInput").ap()
    c4 = nc.dram_tensor("c4", [128, S], BF16, kind="ExternalInput").ap()
    s4 = nc.dram_tensor("s4", [128, S], BF16, kind="ExternalInput").ap()
    maskT = nc.dram_tensor("maskT", [128, 256], BF16,
                           kind="ExternalInput").ap()
    o = nc.dram_tensor("o", [T, D], BF16, kind="ExternalOutput").ap()

    with tile.TileContext(nc) as tc:
        with tc.tile_pool(name="res", bufs=1) as res, \
             tc.tile_pool(name="xtp", bufs=2) as xtp, \
             tc.tile_pool(name="ropet", bufs=2) as rp, \
             tc.tile_pool(name="probs", bufs=4) as probsp, \
             tc.tile_pool(name="attnp", bufs=2) as attnp, \
             tc.tile_pool(name="normp", bufs=2) as normp, \
             tc.tile_pool(name="outp", bufs=2) as outp, \
             tc.tile_pool(name="mmps", bufs=2, space="PSUM") as mmps, \
             tc.tile_pool(name="sps", bufs=2, space="PSUM") as sps, \
             tc.tile_pool(name="pvps", bufs=1, space="PSUM") as pvps:

            ident64 = res.tile([64, 64], BF16)
            make_identity(nc, ident64[:])
            c4_sb = res.tile([128, S], BF16)
            s4_sb = res.tile([128, S], BF16)
            maskT_sb = res.tile([128, 256], BF16)
            maskT3 = maskT_sb.rearrange("p (h c) -> p h c", c=128)

            QRI_A = res.tile([128, T], BF16)   # [h0r h0i h1r h1i] x tokens
            QRI_B = res.tile([128, T], BF16)   # [h2r h2i h3r h3i]
            KRI2 = res.tile([128, T], BF16)    # [Kr Ki Kr Ki]
            Vt_sb = res.tile([128, 32 * 65], BF16)  # kt-tile k at cols k*65
            Vt3 = Vt_sb.rearrange("p (k c) -> p k c", c=65)
            wqkv_r = res.tile([128, 16 * 384], BF16)
            woT_r = res.tile([128, 2 * D], BF16)
            ones32 = res.tile([128, 32], BF16)
            nc.gpsimd.memset(ones32[:], 1.0)
            nc.vector.tensor_copy(Vt3[:, :, 64], ones32[:])
            ones64 = res.tile([1, 64], F32)
            nc.gpsimd.memset(ones64[:], 1.0)
            # ---- DMA plan: Sync queue carries x tiles + o writes + norm
            # reshapes; Scalar queue carries only the weight preload (so
            # phase-2 exps are never stuck behind DMA blocks).
            xt_tiles = {}

            def issue_xt(j):
                xall = xtp.tile([128, 16 * 512], BF16, name="xall")
                if j < 2:
                    # per-tile loads so proj(j)'s d-loop starts asap
                    for d in range(16):
                        nc.sync.dma_start(
                            xall[:, d * 512:(d + 1) * 512],
                            xTt[j * 128:(j + 1) * 128,
                                d * 512:(d + 1) * 512])
                else:
                    # prefetched a pair ahead; one 2MB linear DMA
                    nc.sync.dma_start(
                        xall[:], xTt[j * 128:(j + 1) * 128, :])
                xt_tiles[j] = xall
                if j == 0:
                    for d in range(16):
                        nc.scalar.dma_start(
                            wqkv_r[:, d * 384:(d + 1) * 384],
                            wqkvT[d * 128:(d + 1) * 128, :])
                    nc.scalar.dma_start(c4_sb[:], c4[:])
                    nc.scalar.dma_start(s4_sb[:], s4[:])
                elif j == 1:
                    nc.scalar.dma_start(maskT_sb[:], maskT[:])
                    for t in range(2):
                        nc.scalar.dma_start(woT_r[:, t * D:(t + 1) * D],
                                            woT[t * 128:(t + 1) * 128, :])

            issue_xt(0)

            # ---------------- projection + rope for one 512-token chunk
            def proj(j):
                xall = xt_tiles.pop(j)
                kb = rp.tile([64, 512], BF16, name="kb")
                vsb = rp.tile([64, 512], BF16, name="vsb")
                qrb = rp.tile([128, 512], BF16, name="qrb")
                qib = rp.tile([128, 512], BF16, name="qib")
                for ch in range(3):
                    ps = mmps.tile([128, 512], F32, name="mm")
                    for d in range(16):
                        nc.tensor.matmul(
                            ps[:],
                            wqkv_r[:, d * 384 + ch * 128:
                                   d * 384 + (ch + 1) * 128],
                            xall[:, d * 512:(d + 1) * 512],
                            start=(d == 0), stop=(d == 15))
                    # stage before the next tile() call recycles the slot
                    if ch == 0:
                        nc.scalar.copy(qrb[:], ps[:])
                    elif ch == 1:
                        nc.scalar.copy(qib[:], ps[:])
                    else:
                        nc.scalar.copy(kb[:], ps[0:64, :])
                        nc.scalar.copy(vsb[:], ps[64:128, :])
                    drain_wo(1)
                # V transpose tiles (PE, psum slot shared with proj/wo)
                vtp = mmps.tile([128, 256], BF16, name="vtp", tag="mm")
                for i in range(4):
                    nc.tensor.transpose(
                        vtp[:, i * 64:(i + 1) * 64],
                        vsb[:, i * 128:(i + 1) * 128], ident64[:])
                vtp3 = vtp.rearrange("p (k c) -> p k c", c=64)
                nc.vector.tensor_copy(
                    Vt3[:, j * 4:j * 4 + 4, 0:64], vtp3[:])
                drain_wo(1)

                def rope():
                    tb = j * 512
                    bc = (j % 4) * 512
                    cs = c4_sb[:, bc:bc + 512]
                    sn = s4_sb[:, bc:bc + 512]
                    cs32 = c4_sb[0:32, bc:bc + 512]
                    sn32 = s4_sb[0:32, bc:bc + 512]
                    u1 = rp.tile([32, 512], BF16, name="u1", bufs=1)
                    u2 = rp.tile([32, 512], BF16, name="u2", bufs=1)
                    u3 = rp.tile([32, 512], BF16, name="u3", bufs=1)
                    u4 = rp.tile([32, 512], BF16, name="u4", bufs=1)
                    cs32b = c4_sb[32:64, bc:bc + 512]
                    sn32b = s4_sb[32:64, bc:bc + 512]
                    nc.vector.tensor_mul(u1[:], kb[0:32, :], cs32)
                    nc.vector.tensor_mul(u2[:], kb[32:64, :], sn32b)
                    nc.vector.tensor_mul(u3[:], kb[0:32, :], sn32)
                    nc.vector.tensor_mul(u4[:], kb[32:64, :], cs32b)
                    for g in (0, 64):
                        nc.vector.tensor_sub(
                            KRI2[g:g + 32, tb:tb + 512], u1[:], u2[:])
                    for g in (32, 96):
                        nc.vector.tensor_add(
                            KRI2[g:g + 32, tb:tb + 512], u3[:], u4[:])
                    t1 = rp.tile([128, 512], BF16, name="t1", bufs=1)
                    t2 = rp.tile([128, 512], BF16, name="t2", bufs=1)
                    t3 = rp.tile([128, 512], BF16, name="t3", bufs=1)
                    t4 = rp.tile([128, 512], BF16, name="t4", bufs=1)
                    nc.vector.tensor_mul(t1[:], qrb[:], cs)
                    nc.vector.tensor_mul(t3[:], qrb[:], sn)
                    nc.vector.tensor_mul(t2[:], qib[:], sn)
                    nc.vector.tensor_mul(t4[:], qib[:], cs)
                    for hh in range(4):
                        dst = QRI_A if hh < 2 else QRI_B
                        base = (hh % 2) * 64
                        nc.vector.tensor_sub(
                            dst[base:base + 32, tb:tb + 512],
                            t1[32 * hh:32 * hh + 32, :],
                            t2[32 * hh:32 * hh + 32, :])
                        nc.vector.tensor_add(
                            dst[base + 32:base + 64, tb:tb + 512],
                            t3[32 * hh:32 * hh + 32, :],
                            t4[32 * hh:32 * hh + 32, :])
                return rope

            # ---------------- attention machinery
            pending = deque()
            norm_q = deque()
            nprog = {"enq": 0, "pi_done": 0}
            NPAIRS = 8

            def enqueue_wo(attn01, attn23, qb):
                state = {}
                pid = nprog["enq"]
                nprog["enq"] += 1
                for qs in range(4):
                    for do in range(4):
                        pending.append(
                            (pid, attn01, attn23, qb, qs, do, state))

            def drain_wo(n, reserve=0):
                for _ in range(n):
                    if len(pending) <= reserve:
                        return
                    if pending[0][0] >= nprog["pi_done"] // 2:
                        return   # this pair's attn not normed yet
                    pid, attn01, attn23, qb, qs, do, state = \
                        pending.popleft()
                    last = pid == NPAIRS - 1
                    if do == 0:
                        state[qs] = outp.tile([128, D], BF16, name="osb")
                    osb = state[qs]
                    Ops = mmps.tile([128, 512], F32, name="mm")
                    nc.tensor.matmul(
                        Ops[:], attn01[:, qs * 128:(qs + 1) * 128],
                        woT_r[:, do * 512:(do + 1) * 512],
                        start=True, stop=False)
                    nc.tensor.matmul(
                        Ops[:], attn23[:, qs * 128:(qs + 1) * 128],
                        woT_r[:, D + do * 512:D + (do + 1) * 512],
                        start=False, stop=True)
                    if last and do % 2 == 1:
                        nc.scalar.copy(
                            osb[:, do * 512:(do + 1) * 512], Ops[:])
                    else:
                        nc.vector.tensor_copy(
                            osb[:, do * 512:(do + 1) * 512], Ops[:])
                    if do == 3:
                        qq = qb + qs * 128
                        eng = nc.scalar if (last and qs % 2) else nc.sync
                        eng.dma_start(o[qq:qq + 128, :], osb[:])

            def emit_pv(PVs, b, pkt, ppg, pcsl, stop):
                vt = Vt3[:, b * 16 + pkt, :]
                st = (pkt == 0)
                for hh in range(2):
                    hs = hh * 512
                    nc.tensor.matmul(
                        PVs[:, hs + pcsl.start:hs + pcsl.stop], vt,
                        ppg[:, hs + pcsl.start:hs + pcsl.stop],
                        start=st, stop=stop)

            def make_norm(pvc, pi, attn, pe_bcst=False):
                st = {}
                cb = 1024 * pi

                def s1():
                    st["d8"] = normp.tile([8, 128], F32, name="d8")
                    nc.sync.dma_start(st["d8"][:],
                                      pvc[64:65, cb:cb + 1024])

                def s2():
                    l8 = normp.tile([8, 128], F32, name="l8")
                    nc.scalar.activation(l8[:], st["d8"][:], AF.Ln)
                    st["r8"] = normp.tile([8, 128], F32, name="r8")
                    nc.scalar.activation(st["r8"][:], l8[:],
                                         AF.Exp, scale=-1.0)

                def s3():
                    rec1 = normp.tile([1, 1024], F32, name="rec1")
                    nc.sync.dma_start(rec1[:], st["r8"][:])
                    if pe_bcst:
                        # tail: PE is idle, gpsimd broadcast has ~3us of
                        # fixed overhead -> K=1 outer-product broadcast
                        bcst = sps.tile([64, 1024], F32, name="Sg",
                                        tag="Sg")
                        for hh in range(2):
                            nc.tensor.matmul(
                                bcst[:, 512 * hh:512 * hh + 512],
                                ones64[:],
                                rec1[:, 512 * hh:512 * hh + 512],
                                start=True, stop=True)
                    else:
                        bcst = normp.tile([64, 1024], F32, name="bc")
                        nc.gpsimd.partition_broadcast(bcst[:], rec1[:])
                    for hh in range(2):
                        nc.vector.tensor_mul(
                            attn[64 * hh:64 * hh + 64, :],
                            pvc[0:64, cb + 512 * hh:cb + 512 * hh + 512],
                            bcst[:, 512 * hh:512 * hh + 512])
                    nprog["pi_done"] += 1
                return [s1, s2, s3]

            def attention(b, jp):
                qb = b * S + jp * 512
                nkt = 4 * jp + 4
                # during the final pair, hold back a few normed wo steps so
                # the tail's norm-latency window has PE work queued
                rsv = 6 if (b, jp) == (1, 3) else 0
                attn01 = attnp.tile([128, 512], BF16, name="at01")
                attn23 = attnp.tile([128, 512], BF16, name="at23")
                pvc = normp.tile([65, 2048], F32, name="pvc")
                for pi, (QRI, attn) in enumerate(
                        ((QRI_A, attn01), (QRI_B, attn23))):
                    drain_wo(2, rsv)
                    PVs = pvps.tile([65, 1024], F32, name="PV")
                    pgq = deque()
                    for kt in range(nkt):
                        kc = b * S + kt * 128
                        r = kt - (nkt - 4)
                        cs0 = max(0, 128 * r)   # live q-col start
                        csl = slice(cs0, 512)
                        Sg = sps.tile([128, 1024], F32, name="Sg")
                        pg = probsp.tile([128, 1024], BF16, name="pg")
                        for hh in range(2):
                            hs = hh * 512
                            nc.tensor.matmul(
                                Sg[:, hs + cs0:hs + 512],
                                KRI2[64 * hh:64 * hh + 64, kc:kc + 128],
                                QRI[64 * hh:64 * hh + 64,
                                    qb + cs0:qb + 512],
                                start=True, stop=True,
                                tile_position=(64 * hh, 0))
                        sgv = Sg.rearrange(
                            "p (h c) -> p h c", h=2)[:, :, cs0:512]
                        pgv = pg.rearrange(
                            "p (h c) -> p h c", h=2)[:, :, cs0:512]
                        nc.scalar.activation(pgv, sgv, AF.Exp, scale=0.125)
                        if r >= 0:
                            pgt = pg.rearrange(
                                "p (h c) -> p h c",
                                h=2)[:, :, cs0:cs0 + 128]
                            nc.vector.tensor_mul(pgt, pgt, maskT3[:])
                        pgq.append((kt, pg, csl))
                        if len(pgq) > 2:
                            emit_pv(PVs, b, *pgq.popleft(), stop=False)
                        if kt in (1, 2, 3) and norm_q:
                            norm_q.popleft()()
                        drain_wo(2 if len(pending) > 8 else 1, rsv)
                    while pgq:
                        item = pgq.popleft()
                        emit_pv(PVs, b, *item, stop=not pgq)
                    drain_wo(2, rsv)
                    # free the PV banks asap; norm (per pi) is deferred
                    # into the following kt loops
                    nc.vector.tensor_copy(
                        pvc[:, 1024 * pi:1024 * pi + 1024], PVs[:])
                    norm_q.extend(make_norm(pvc, pi, attn,
                                            pe_bcst=rsv > 0))
                enqueue_wo(attn01, attn23, qb)

            # ---------------- fused emission stream
            stream = [("p", 0), ("p", 1), ("a", 0, 0), ("p", 2),
                      ("a", 0, 1), ("p", 3), ("a", 0, 2), ("p", 4),
                      ("a", 0, 3), ("p", 5), ("a", 1, 0), ("p", 6),
                      ("a", 1, 1), ("p", 7), ("a", 1, 2), ("a", 1, 3)]
            next_xt = 2
            rope_pend = None
            for step in stream:
                if step[0] == "p":
                    j = step[1]
                    rope_fn = proj(j)
                    if j < 2:
                        # needed by the very next pair; emit inline
                        rope_fn()
                    else:
                        rope_pend = rope_fn
                    if j == 0:
                        issue_xt(1)
                else:
                    if next_xt < 8:
                        issue_xt(next_xt)
                        next_xt += 1
                    attention(step[1], step[2])
                    # rope for chunk j lands after the pair that follows
                    # proj(j), so it never delays that pair's norm muls
                    if rope_pend is not None:
                        rope_pend()
                        rope_pend = None
            # emit any ungated wo steps (the last pair's reserve) BEFORE
            # the final norm chain so the PE has work during its latency
            drain_wo(1 << 30)
            while norm_q:
                norm_q.popleft()()
            drain_wo(1 << 30)

    nc.compile()
    bacc.get_activation_tables = _orig_gat
    return nc


def _prep_inputs(x, freqs_cos, freqs_sin, wq, wk, wv, wo):
    from ml_dtypes import bfloat16
    xf = np.asarray(x, np.float32).reshape(T, D)
    xTf = np.ascontiguousarray(xf.T).astype(bfloat16)      # [D, T]
    # j-major p-row-major layout: xTt[j, p, d, c] = xT[d*128+p, j*512+c]
    xTt = np.ascontiguousarray(
        xTf.reshape(16, 128, 8, 512).transpose(2, 1, 0, 3)
    ).reshape(8 * 128, 16 * 512)
    wq = np.asarray(wq, np.float32)
    wk = np.asarray(wk, np.float32)
    wv = np.asarray(wv, np.float32)
    wo = np.asarray(wo, np.float32)
    fc = np.asarray(freqs_cos, np.float32)
    fs = np.asarray(freqs_sin, np.float32)

    c4 = np.ascontiguousarray(np.tile(fc.T, (4, 1))).astype(bfloat16)
    s4 = np.ascontiguousarray(np.tile(fs.T, (4, 1))).astype(bfloat16)
    kt = np.arange(128)[:, None]
    qt = np.arange(128)[None, :]
    tri = (kt <= qt).astype(np.float32)
    maskT = np.ascontiguousarray(np.tile(tri, (1, 2))).astype(bfloat16)
    ev = np.arange(0, 64, 2)
    od = np.arange(1, 64, 2)

    in_maps = []
    for c in range(NCORES):
        qreal = np.concatenate([(4 * c + h) * 64 + ev for h in range(4)])
        qimag = np.concatenate([(4 * c + h) * 64 + od for h in range(4)])
        Wc = np.concatenate([wq[qreal], wq[qimag], wk[c * 64 + ev],
                             wk[c * 64 + od], wv[c * 64:(c + 1) * 64]], axis=0)
        in_maps.append({
            "xTt": xTt,
            "wqkvT": np.ascontiguousarray(Wc.T).astype(bfloat16),
            "woT": np.ascontiguousarray(
                wo[:, c * 256:(c + 1) * 256].T).astype(bfloat16),
            "c4": c4, "s4": s4, "maskT": maskT,
        })
    return in_maps


def _run(in_maps, trace=False, **kw):
    from concourse import bass_utils
    if "nc" not in _cache:
        _cache["nc"] = _build()
    return bass_utils.run_bass_kernel_spmd(
        _cache["nc"], in_maps, core_ids=list(range(NCORES)), trace=trace, **kw)


def kernel(x, freqs_cos, freqs_sin, wq, wk, wv, wo):
    in_maps = _prep_inputs(x, freqs_cos, freqs_sin, wq, wk, wv, wo)
    res = _run(in_maps)
    out = np.zeros((T, D), np.float64)
    for c in range(NCORES):
        out += np.asarray(res.results[c]["o"], np.float32)
    return out.astype(np.float32).reshape(B, S, D)


# revision 34
# speedup vs baseline: 1.3713x; 1.0414x over previous
"""Trainium2 Bass kernel for nn_Attention_14542759264705.

Dense transformer attention: QKV proj + interleaved RoPE + GQA causal
attention (32 q heads / 8 kv heads, hd=64) + output proj, fp32 in/out.

Sharding: tensor-parallel over kv-head groups across 8 cores. Core c owns
q heads 4c..4c+3 and kv head c; each core computes a partial output and
the host sums the 8 partials.

v3 (vs the v2 two-phase baseline):
  - Single fused PE stream: projection j-chunks are interleaved with
    attention pairs (proj0, proj1, A00, proj2, A01, ... A13) so the PE
    never parks at phase/pair boundaries (v2 lost ~146us to HAM
    re-throttling during starvation windows).
  - Warm-up matmuls at t=0 (no DMA deps) lift the PE clock gate before
    the first real matmul's data lands.
  - Dual-queue DMA: x tiles + weights alternate between the Sync and
    Scalar hardware-DGE queues (~2x ingest); x is tile-contiguous in
    DRAM so each [128,512] tile is one linear 128KB read.
  - Causal trim at 128-col granularity (v2 trimmed at 256) for scores,
    exp, and PV; tri-mask multiply is a single [128,2,128] DVE op.
  - Softmax denominators: [1,2048] row is DMA-reshaped to [16,128], so
    the Ln/Exp reciprocal costs ~0.7us of ACT instead of 4.6us.
  - wo psum->sbuf casts alternate DVE/ACT to split the ~100us cast load.
  - Projection channels run sequentially through one [128,512] psum slot
    pool (shared with wo + V-transposes) so proj+attention coexist in
    the 8 PSUM banks.
"""
import numpy as np

B, S, D = 2, 2048, 2048
T = B * S
NH, NKV, HD = 32, 8, 64
NCORES = 8

_cache = {}


def _build():
    from collections import deque
    from itertools import cycle

    import concourse.bacc as bacc
    import concourse.mybir as mybir
    import concourse.tile as tile
    from concourse.masks import make_identity

    F32 = mybir.dt.float32
    BF16 = mybir.dt.bfloat16
    AF = mybir.ActivationFunctionType

    # Force Exp/Ln/Copy onto the single combined act table set so the
    # compiler never inserts per-call ACT_TABLE_LOADs between exps and
    # the Ln/Exp reciprocal.
    from concourse.hw_specs import get_activation_tables as _gat

    def _patched_tables(arch):
        tabs = _gat(arch)
        key = "natural_log_exp_and_others"
        comb = tabs[key]
        return {n: (s if n == key else (s - comb)) for n, s in tabs.items()}

    _orig_gat = bacc.get_activation_tables
    bacc.get_activation_tables = _patched_tables

    nc = bacc.Bacc("TRN2", target_bir_lowering=False, debug=False,
                   num_devices=NCORES)
    # x stored j-major, partition-row major: xTt[j*128+p, d*512+c] holds
    # x^T element (d*128+p, j*512+c) -> each j-chunk is one plain 2D DMA
    xTt = nc.dram_tensor("xTt", [8 * 128, 16 * 512], BF16,
                         kind="ExternalInput").ap()
    wqkvT = nc.dram_tensor("wqkvT", [D, 384], BF16, kind="ExternalInput").ap()
    woT = nc.dram_tensor("woT", [256, D], BF16, kind="ExternalInput").ap()
    c4 = nc.dram_tensor("c4", [128, S], BF16, kind="ExternalInput").ap()
    s4 = nc.dram_tensor("s4", [128, S], BF16, kind="ExternalInput").ap()
    maskT = nc.dram_tensor("maskT", [128, 256], BF16,
                           kind="ExternalInput").ap()
    o = nc.dram_tensor("o", [T, D], BF16, kind="ExternalOutput").ap()

    with tile.TileContext(nc) as tc:
        with tc.tile_pool(name="res", bufs=1) as res, \
             tc.tile_pool(name="xtp", bufs=2) as xtp, \
             tc.tile_pool(name="ropet", bufs=2) as rp, \
             tc.tile_pool(name="probs", bufs=4) as probsp, \
             tc.tile_pool(name="attnp", bufs=2) as attnp, \
             tc.tile_pool(name="normp", bufs=2) as normp, \
             tc.tile_pool(name="outp", bufs=2) as outp, \
             tc.tile_pool(name="mmps", bufs=2, space="PSUM") as mmps, \
             tc.tile_pool(name="sps", bufs=2, space="PSUM") as sps, \
             tc.tile_pool(name="pvps", bufs=1, space="PSUM") as pvps:

            ident64 = res.tile([64, 64], BF16)
            make_identity(nc, ident64[:])
            c4_sb = res.tile([128, S], BF16)
            s4_sb = res.tile([128, S], BF16)
            maskT_sb = res.tile([128, 256], BF16)
            maskT3 = maskT_sb.rearrange("p (h c) -> p h c", c=128)

            QRI_A = res.tile([128, T], BF16)   # [h0r h0i h1r h1i] x tokens
            QRI_B = res.tile([128, T], BF16)   # [h2r h2i h3r h3i]
            KRI2 = res.tile([128, T], BF16)    # [Kr Ki Kr Ki]
            Vt_sb = res.tile([128, 32 * 65], BF16)  # kt-tile k at cols k*65
            Vt3 = Vt_sb.rearrange("p (k c) -> p k c", c=65)
            wqkv_r = res.tile([128, 16 * 384], BF16)
            woT_r = res.tile([128, 2 * D], BF16)
            ones32 = res.tile([128, 32], BF16)
            nc.gpsimd.memset(ones32[:], 1.0)
            nc.vector.tensor_copy(Vt3[:, :, 64], ones32[:])
            ones64 = res.tile([1, 64], BF16)
            nc.gpsimd.memset(ones64[:], 1.0)
            warmM = res.tile([128, 512], BF16)
            nc.gpsimd.memset(warmM[:], 0.0)

            # PE warm-up: real matmuls are DMA-paced until ~16us, so a
            # ~4us warm burst lifts the HAM clock gate to 2.4GHz right
            # as the first projection matmuls issue.
            for _ in range(10):
                wps = sps.tile([128, 1024], F32, name="Sg")
                nc.tensor.matmul(wps[:, 0:512], warmM[:, 0:128], warmM[:],
                                 start=True, stop=True)
            # ---- DMA plan: Sync queue carries x tiles + o writes + norm
            # reshapes; Scalar queue carries only the weight preload (so
            # phase-2 exps are never stuck behind DMA blocks).
            xt_tiles = {}

            def issue_xt(j):
                xall = xtp.tile([128, 16 * 512], BF16, name="xall")
                if j < 2:
                    # per-tile loads so proj(j)'s d-loop starts asap
                    for d in range(16):
                        nc.sync.dma_start(
                            xall[:, d * 512:(d + 1) * 512],
                            xTt[j * 128:(j + 1) * 128,
                                d * 512:(d + 1) * 512])
                else:
                    # prefetched a pair ahead; one 2MB linear DMA
                    nc.sync.dma_start(
                        xall[:], xTt[j * 128:(j + 1) * 128, :])
                xt_tiles[j] = xall
                if j == 0:
                    for d in range(16):
                        nc.scalar.dma_start(
                            wqkv_r[:, d * 384:(d + 1) * 384],
                            wqkvT[d * 128:(d + 1) * 128, :])
                    nc.scalar.dma_start(c4_sb[:], c4[:])
                    nc.scalar.dma_start(s4_sb[:], s4[:])
                elif j == 1:
                    nc.sync.dma_start(maskT_sb[:], maskT[:])
                    for t in range(2):
                        nc.sync.dma_start(woT_r[:, t * D:(t + 1) * D],
                                          woT[t * 128:(t + 1) * 128, :])

            issue_xt(0)

            # ---------------- projection + rope for one 512-token chunk
            def proj(j):
                xall = xt_tiles.pop(j)
                kb = rp.tile([64, 512], BF16, name="kb")
                vsb = rp.tile([64, 512], BF16, name="vsb")
                qrb = rp.tile([128, 512], BF16, name="qrb")
                qib = rp.tile([128, 512], BF16, name="qib")
                for ch in range(3):
                    ps = mmps.tile([128, 512], F32, name="mm")
                    for d in range(16):
                        nc.tensor.matmul(
                            ps[:],
                            wqkv_r[:, d * 384 + ch * 128:
                                   d * 384 + (ch + 1) * 128],
                            xall[:, d * 512:(d + 1) * 512],
                            start=(d == 0), stop=(d == 15))
                    # stage before the next tile() call recycles the slot
                    if ch == 0:
                        nc.scalar.copy(qrb[:], ps[:])
                    elif ch == 1:
                        nc.scalar.copy(qib[:], ps[:])
                    else:
                        nc.scalar.copy(kb[:], ps[0:64, :])
                        nc.scalar.copy(vsb[:], ps[64:128, :])
                    drain_wo(1)
                # V transpose tiles (PE, psum slot shared with proj/wo)
                vtp = mmps.tile([128, 256], BF16, name="vtp", tag="mm")
                for i in range(4):
                    nc.tensor.transpose(
                        vtp[:, i * 64:(i + 1) * 64],
                        vsb[:, i * 128:(i + 1) * 128], ident64[:])
                vtp3 = vtp.rearrange("p (k c) -> p k c", c=64)
                nc.vector.tensor_copy(
                    Vt3[:, j * 4:j * 4 + 4, 0:64], vtp3[:])
                drain_wo(1)

                def rope():
                    tb = j * 512
                    bc = (j % 4) * 512
                    cs = c4_sb[:, bc:bc + 512]
                    sn = s4_sb[:, bc:bc + 512]
                    cs32 = c4_sb[0:32, bc:bc + 512]
                    sn32 = s4_sb[0:32, bc:bc + 512]
                    u1 = rp.tile([32, 512], BF16, name="u1", bufs=1)
                    u2 = rp.tile([32, 512], BF16, name="u2", bufs=1)
                    u3 = rp.tile([32, 512], BF16, name="u3", bufs=1)
                    u4 = rp.tile([32, 512], BF16, name="u4", bufs=1)
                    cs32b = c4_sb[32:64, bc:bc + 512]
                    sn32b = s4_sb[32:64, bc:bc + 512]
                    nc.vector.tensor_mul(u1[:], kb[0:32, :], cs32)
                    nc.vector.tensor_mul(u2[:], kb[32:64, :], sn32b)
                    nc.vector.tensor_mul(u3[:], kb[0:32, :], sn32)
                    nc.vector.tensor_mul(u4[:], kb[32:64, :], cs32b)
                    for g in (0, 64):
                        nc.vector.tensor_sub(
                            KRI2[g:g + 32, tb:tb + 512], u1[:], u2[:])
                    for g in (32, 96):
                        nc.vector.tensor_add(
                            KRI2[g:g + 32, tb:tb + 512], u3[:], u4[:])
                    t1 = rp.tile([128, 512], BF16, name="t1", bufs=1)
                    t2 = rp.tile([128, 512], BF16, name="t2", bufs=1)
                    t3 = rp.tile([128, 512], BF16, name="t3", bufs=1)
                    t4 = rp.tile([128, 512], BF16, name="t4", bufs=1)
                    nc.vector.tensor_mul(t1[:], qrb[:], cs)
                    nc.vector.tensor_mul(t3[:], qrb[:], sn)
                    nc.vector.tensor_mul(t2[:], qib[:], sn)
                    nc.vector.tensor_mul(t4[:], qib[:], cs)
                    for hh in range(4):
                        dst = QRI_A if hh < 2 else QRI_B
                        base = (hh % 2) * 64
                        nc.vector.tensor_sub(
                            dst[base:base + 32, tb:tb + 512],
                            t1[32 * hh:32 * hh + 32, :],
                            t2[32 * hh:32 * hh + 32, :])
                        nc.vector.tensor_add(
                            dst[base + 32:base + 64, tb:tb + 512],
                            t3[32 * hh:32 * hh + 32, :],
                            t4[32 * hh:32 * hh + 32, :])
                return rope

            # ---------------- attention machinery
            pending = deque()
            norm_q = deque()
            nprog = {"enq": 0, "pi_done": 0}
            NPAIRS = 8

            def enqueue_wo(attn01, attn23, qb):
                state = {}
                pid = nprog["enq"]
                nprog["enq"] += 1
                for qs in range(4):
                    for do in range(4):
                        pending.append(
                            (pid, attn01, attn23, qb, qs, do, state))

            def drain_wo(n, reserve=0):
                for _ in range(n):
                    if len(pending) <= reserve:
                        return
                    if pending[0][0] >= nprog["pi_done"] // 2:
                        return   # this pair's attn not normed yet
                    pid, attn01, attn23, qb, qs, do, state = \
                        pending.popleft()
                    last = pid == NPAIRS - 1
                    if do == 0:
                        state[qs] = outp.tile([128, D], BF16, name="osb")
                    osb = state[qs]
                    Ops = mmps.tile([128, 512], F32, name="mm")
                    nc.tensor.matmul(
                        Ops[:], attn01[:, qs * 128:(qs + 1) * 128],
                        woT_r[:, do * 512:(do + 1) * 512],
                        start=True, stop=False)
                    nc.tensor.matmul(
                        Ops[:], attn23[:, qs * 128:(qs + 1) * 128],
                        woT_r[:, D + do * 512:D + (do + 1) * 512],
                        start=False, stop=True)
                    if last and do % 2 == 1:
                        nc.scalar.copy(
                            osb[:, do * 512:(do + 1) * 512], Ops[:])
                    else:
                        nc.vector.tensor_copy(
                            osb[:, do * 512:(do + 1) * 512], Ops[:])
                    if do == 3:
                        qq = qb + qs * 128
                        eng = nc.scalar if (last and qs % 2) else nc.sync
                        eng.dma_start(o[qq:qq + 128, :], osb[:])

            def emit_pv(PVs, b, pkt, ppg, pcsl, stop):
                vt = Vt3[:, b * 16 + pkt, :]
                st = (pkt == 0)
                for hh in range(2):
                    hs = hh * 512
                    nc.tensor.matmul(
                        PVs[:, hs + pcsl.start:hs + pcsl.stop], vt,
                        ppg[:, hs + pcsl.start:hs + pcsl.stop],
                        start=st, stop=stop)

            def make_norm(pvc, pi, attn, pe_bcst=False):
                st = {}
                cb = 1024 * pi

                def s1():
                    st["d8"] = normp.tile([8, 128], F32, name="d8")
                    nc.sync.dma_start(st["d8"][:],
                                      pvc[64:65, cb:cb + 1024])

                def s2():
                    l8 = normp.tile([8, 128], F32, name="l8")
                    nc.scalar.activation(l8[:], st["d8"][:], AF.Ln)
                    st["r8"] = normp.tile([8, 128], BF16, name="r8")
                    nc.scalar.activation(st["r8"][:], l8[:],
                                         AF.Exp, scale=-1.0)

                def s3():
                    rec1 = normp.tile([1, 1024], BF16, name="rec1")
                    nc.sync.dma_start(rec1[:], st["r8"][:])
                    if pe_bcst:
                        # tail: PE is idle, gpsimd broadcast has ~3us of
                        # fixed overhead -> K=1 outer-product broadcast
                        # (bf16 single-pass; bf16 recip costs ~0.2% rel)
                        bcst = sps.tile([64, 1024], F32, name="Sg",
                                        tag="Sg")
                        for hh in range(2):
                            nc.tensor.matmul(
                                bcst[:, 512 * hh:512 * hh + 512],
                                ones64[:],
                                rec1[:, 512 * hh:512 * hh + 512],
                                start=True, stop=True)
                    else:
                        bcst = normp.tile([64, 1024], BF16, name="bc")
                        nc.gpsimd.partition_broadcast(bcst[:], rec1[:])
                    for hh in range(2):
                        nc.vector.tensor_mul(
                            attn[64 * hh:64 * hh + 64, :],
                            pvc[0:64, cb + 512 * hh:cb + 512 * hh + 512],
                            bcst[:, 512 * hh:512 * hh + 512])
                    nprog["pi_done"] += 1
                return [s1, s2, s3]

            def attention(b, jp, rope_mid=None):
                qb = b * S + jp * 512
                nkt = 4 * jp + 4
                # during the final pair, hold back a few normed wo steps so
                # the tail's norm-latency window has PE work queued
                rsv = 6 if (b, jp) == (1, 3) else 0
                attn01 = attnp.tile([128, 512], BF16, name="at01")
                attn23 = attnp.tile([128, 512], BF16, name="at23")
                pvc = normp.tile([65, 2048], F32, name="pvc")
                QRIs = (QRI_A, QRI_B)
                attns = (attn01, attn23)
                pgqs = (deque(), deque())

                def emit_score(pi, kt):
                    kc = b * S + kt * 128
                    r = kt - (nkt - 4)
                    cs0 = max(0, 128 * r)   # live q-col start
                    Sg = sps.tile([128, 1024], F32, name="Sg")
                    pg = probsp.tile([128, 1024], BF16, name="pg")
                    for hh in range(2):
                        hs = hh * 512
                        nc.tensor.matmul(
                            Sg[:, hs + cs0:hs + 512],
                            KRI2[64 * hh:64 * hh + 64, kc:kc + 128],
                            QRIs[pi][64 * hh:64 * hh + 64,
                                     qb + cs0:qb + 512],
                            start=True, stop=True,
                            tile_position=(64 * hh, 0))
                    sgv = Sg.rearrange(
                        "p (h c) -> p h c", h=2)[:, :, cs0:512]
                    pgv = pg.rearrange(
                        "p (h c) -> p h c", h=2)[:, :, cs0:512]
                    nc.scalar.activation(pgv, sgv, AF.Exp, scale=0.125)
                    if r >= 0:
                        pgt = pg.rearrange(
                            "p (h c) -> p h c", h=2)[:, :, cs0:cs0 + 128]
                        nc.vector.tensor_mul(pgt, pgt, maskT3[:])
                    pgqs[pi].append((kt, pg, slice(cs0, 512)))

                def finish_pi(pi, PVs):
                    pgq = pgqs[pi]
                    while pgq:
                        item = pgq.popleft()
                        emit_pv(PVs, b, *item, stop=not pgq)
                        drain_wo(1, rsv)
                    if norm_q:
                        norm_q.popleft()()
                    # free the PV banks asap; norm (per pi) is deferred
                    # into the following kt loops
                    nc.vector.tensor_copy(
                        pvc[:, 1024 * pi:1024 * pi + 1024], PVs[:])
                    norm_q.extend(make_norm(pvc, pi, attns[pi],
                                            pe_bcst=rsv > 0))

                # ---- pi0 ----
                drain_wo(2, rsv)
                PVs0 = pvps.tile([65, 1024], F32, name="PV")
                for kt in range(nkt):
                    emit_score(0, kt)
                    if len(pgqs[0]) > 2:
                        emit_pv(PVs0, b, *pgqs[0].popleft(), stop=False)
                    if kt in (1, 2, 3) and norm_q:
                        norm_q.popleft()()
                    drain_wo(2 if len(pending) > 8 else 1, rsv)
                # hoist pi1's first scores to cover pi0's exp tail
                emit_score(1, 0)
                drain_wo(1, rsv)
                emit_score(1, 1)
                finish_pi(0, PVs0)
                if rope_mid is not None:
                    rope_mid()
                # ---- pi1 ----
                PVs1 = pvps.tile([65, 1024], F32, name="PV")
                for kt in range(2, nkt):
                    emit_score(1, kt)
                    if len(pgqs[1]) > 2:
                        emit_pv(PVs1, b, *pgqs[1].popleft(), stop=False)
                    if kt in (2, 3, 4) and norm_q:
                        norm_q.popleft()()
                    drain_wo(2 if len(pending) > 8 else 1, rsv)
                finish_pi(1, PVs1)
                drain_wo(2, rsv)
                enqueue_wo(attn01, attn23, qb)

            # ---------------- fused emission stream
            stream = [("p", 0), ("p", 1), ("a", 0, 0), ("p", 2),
                      ("a", 0, 1), ("p", 3), ("a", 0, 2), ("p", 4),
                      ("a", 0, 3), ("p", 5), ("a", 1, 0), ("p", 6),
                      ("a", 1, 1), ("p", 7), ("a", 1, 2), ("a", 1, 3)]
            next_xt = 2
            rope_pend = None
            for step in stream:
                if step[0] == "p":
                    j = step[1]
                    rope_fn = proj(j)
                    if j < 2:
                        # needed by the very next pair; emit inline
                        rope_fn()
                    else:
                        rope_pend = rope_fn
                    if j == 0:
                        issue_xt(1)
                else:
                    if next_xt < 8:
                        issue_xt(next_xt)
                        next_xt += 1
                    # rope for chunk j is emitted mid-pair (between pi0
                    # and pi1) so it never sits ahead of the pair's norm
                    # muls or next pair's masks on the DVE queue
                    attention(step[1], step[2], rope_mid=rope_pend)
                    rope_pend = None
            # emit any ungated wo steps (the last pair's reserve) BEFORE
            # the final norm chain so the PE has work during its latency
            drain_wo(1 << 30)
            while norm_q:
                norm_q.popleft()()
            drain_wo(1 << 30)

    nc.compile()
    bacc.get_activation_tables = _orig_gat
    return nc


def _prep_inputs(x, freqs_cos, freqs_sin, wq, wk, wv, wo):
    from ml_dtypes import bfloat16
    xf = np.asarray(x, np.float32).reshape(T, D)
    xTf = np.ascontiguousarray(xf.T).astype(bfloat16)      # [D, T]
    # j-major p-row-major layout: xTt[j, p, d, c] = xT[d*128+p, j*512+c]
    xTt = np.ascontiguousarray(
        xTf.reshape(16, 128, 8, 512).transpose(2, 1, 0, 3)
    ).reshape(8 * 128, 16 * 512)
    wq = np.asarray(wq, np.float32)
    wk = np.asarray(wk, np.float32)
    wv = np.asarray(wv, np.float32)
    wo = np.asarray(wo, np.float32)
    fc = np.asarray(freqs_cos, np.float32)
    fs = np.asarray(freqs_sin, np.float32)

    c4 = np.ascontiguousarray(np.tile(fc.T, (4, 1))).astype(bfloat16)
    s4 = np.ascontiguousarray(np.tile(fs.T, (4, 1))).astype(bfloat16)
    kt = np.arange(128)[:, None]
    qt = np.arange(128)[None, :]
    tri = (kt <= qt).astype(np.float32)
    maskT = np.ascontiguousarray(np.tile(tri, (1, 2))).astype(bfloat16)
    ev = np.arange(0, 64, 2)
    od = np.arange(1, 64, 2)

    in_maps = []
    for c in range(NCORES):
        qreal = np.concatenate([(4 * c + h) * 64 + ev for h in range(4)])
        qimag = np.concatenate([(4 * c + h) * 64 + od for h in range(4)])
        Wc = np.concatenate([wq[qreal], wq[qimag], wk[c * 64 + ev],
                             wk[c * 64 + od], wv[c * 64:(c + 1) * 64]], axis=0)
        in_maps.append({
            "xTt": xTt,
            "wqkvT": np.ascontiguousarray(Wc.T).astype(bfloat16),
            "woT": np.ascontiguousarray(
                wo[:, c * 256:(c + 1) * 256].T).astype(bfloat16),
            "c4": c4, "s4": s4, "maskT": maskT,
        })
    return in_maps


def _run(in_maps, trace=False, **kw):
    from concourse import bass_utils
    if "nc" not in _cache:
        _cache["nc"] = _build()
    return bass_utils.run_bass_kernel_spmd(
        _cache["nc"], in_maps, core_ids=list(range(NCORES)), trace=trace, **kw)


def kernel(x, freqs_cos, freqs_sin, wq, wk, wv, wo):
    in_maps = _prep_inputs(x, freqs_cos, freqs_sin, wq, wk, wv, wo)
    res = _run(in_maps)
    out = np.zeros((T, D), np.float64)
    for c in range(NCORES):
        out += np.asarray(res.results[c]["o"], np.float32)
    return out.astype(np.float32).reshape(B, S, D)


# revision 39
# speedup vs baseline: 1.4021x; 1.0224x over previous
"""Trainium2 Bass kernel for nn_Attention_14542759264705.

Dense transformer attention: QKV proj + interleaved RoPE + GQA causal
attention (32 q heads / 8 kv heads, hd=64) + output proj, fp32 in/out.

Sharding: tensor-parallel over kv-head groups across 8 cores. Core c owns
q heads 4c..4c+3 and kv head c; each core computes a partial output and
the host sums the 8 partials.

v3 (vs the v2 two-phase baseline):
  - Single fused PE stream: projection j-chunks are interleaved with
    attention pairs (proj0, proj1, A00, proj2, A01, ... A13) so the PE
    never parks at phase/pair boundaries (v2 lost ~146us to HAM
    re-throttling during starvation windows).
  - Warm-up matmuls at t=0 (no DMA deps) lift the PE clock gate before
    the first real matmul's data lands.
  - Dual-queue DMA: x tiles + weights alternate between the Sync and
    Scalar hardware-DGE queues (~2x ingest); x is tile-contiguous in
    DRAM so each [128,512] tile is one linear 128KB read.
  - Causal trim at 128-col granularity (v2 trimmed at 256) for scores,
    exp, and PV; tri-mask multiply is a single [128,2,128] DVE op.
  - Softmax denominators: [1,2048] row is DMA-reshaped to [16,128], so
    the Ln/Exp reciprocal costs ~0.7us of ACT instead of 4.6us.
  - wo psum->sbuf casts alternate DVE/ACT to split the ~100us cast load.
  - Projection channels run sequentially through one [128,512] psum slot
    pool (shared with wo + V-transposes) so proj+attention coexist in
    the 8 PSUM banks.
"""
import numpy as np

B, S, D = 2, 2048, 2048
T = B * S
NH, NKV, HD = 32, 8, 64
NCORES = 8

_cache = {}


def _build():
    from collections import deque
    from itertools import cycle

    import concourse.bacc as bacc
    import concourse.mybir as mybir
    import concourse.tile as tile
    from concourse.masks import make_identity

    F32 = mybir.dt.float32
    BF16 = mybir.dt.bfloat16
    AF = mybir.ActivationFunctionType

    # Force Exp/Ln/Copy onto the single combined act table set so the
    # compiler never inserts per-call ACT_TABLE_LOADs between exps and
    # the Ln/Exp reciprocal.
    from concourse.hw_specs import get_activation_tables as _gat

    def _patched_tables(arch):
        tabs = _gat(arch)
        key = "natural_log_exp_and_others"
        comb = tabs[key]
        return {n: (s if n == key else (s - comb)) for n, s in tabs.items()}

    _orig_gat = bacc.get_activation_tables
    bacc.get_activation_tables = _patched_tables

    nc = bacc.Bacc("TRN2", target_bir_lowering=False, debug=False,
                   num_devices=NCORES)
    # x stored j-major, partition-row major: xTt[j*128+p, d*512+c] holds
    # x^T element (d*128+p, j*512+c) -> each j-chunk is one plain 2D DMA
    xTt = nc.dram_tensor("xTt", [8 * 128, 16 * 512], BF16,
                         kind="ExternalInput").ap()
    wqkvT = nc.dram_tensor("wqkvT", [D, 384], BF16, kind="ExternalInput").ap()
    woT = nc.dram_tensor("woT", [256, D], BF16, kind="ExternalInput").ap()
    c4 = nc.dram_tensor("c4", [128, S], BF16, kind="ExternalInput").ap()
    s4 = nc.dram_tensor("s4", [128, S], BF16, kind="ExternalInput").ap()
    maskT = nc.dram_tensor("maskT", [128, 256], BF16,
                           kind="ExternalInput").ap()
    o = nc.dram_tensor("o", [T, D], BF16, kind="ExternalOutput").ap()

    with tile.TileContext(nc) as tc:
        with tc.tile_pool(name="res", bufs=1) as res, \
             tc.tile_pool(name="xtp", bufs=2) as xtp, \
             tc.tile_pool(name="ropet", bufs=2) as rp, \
             tc.tile_pool(name="probs", bufs=4) as probsp, \
             tc.tile_pool(name="attnp", bufs=2) as attnp, \
             tc.tile_pool(name="normp", bufs=2) as normp, \
             tc.tile_pool(name="outp", bufs=2) as outp, \
             tc.tile_pool(name="mmps", bufs=2, space="PSUM") as mmps, \
             tc.tile_pool(name="sps", bufs=2, space="PSUM") as sps, \
             tc.tile_pool(name="pvps", bufs=1, space="PSUM") as pvps:

            ident64 = res.tile([64, 64], BF16)
            make_identity(nc, ident64[:])
            c4_sb = res.tile([128, S], BF16)
            s4_sb = res.tile([128, S], BF16)
            maskT_sb = res.tile([128, 256], BF16)
            maskT3 = maskT_sb.rearrange("p (h c) -> p h c", c=128)

            QRI_A = res.tile([128, T], BF16)   # [h0r h0i h1r h1i] x tokens
            QRI_B = res.tile([128, T], BF16)   # [h2r h2i h3r h3i]
            KRI2 = res.tile([128, T], BF16)    # [Kr Ki Kr Ki]
            Vt_sb = res.tile([128, 32 * 65], BF16)  # kt-tile k at cols k*65
            Vt3 = Vt_sb.rearrange("p (k c) -> p k c", c=65)
            wqkv_r = res.tile([128, 16 * 384], BF16)
            woT_r = res.tile([128, 2 * D], BF16)
            ones32 = res.tile([128, 32], BF16)
            nc.gpsimd.memset(ones32[:], 1.0)
            nc.vector.tensor_copy(Vt3[:, :, 64], ones32[:])
            ones64 = res.tile([1, 64], BF16)
            nc.gpsimd.memset(ones64[:], 1.0)
            warmM = res.tile([128, 512], BF16)
            nc.gpsimd.memset(warmM[:], 0.0)

            # PE warm-up: real matmuls are DMA-paced until ~16us, so a
            # ~4us warm burst lifts the HAM clock gate to 2.4GHz right
            # as the first projection matmuls issue.
            for _ in range(10):
                wps = sps.tile([128, 1024], F32, name="Sg")
                nc.tensor.matmul(wps[:, 0:512], warmM[:, 0:128], warmM[:],
                                 start=True, stop=True)
            # ---- DMA plan: Sync queue carries x tiles + o writes + norm
            # reshapes; Scalar queue carries only the weight preload (so
            # phase-2 exps are never stuck behind DMA blocks).
            xt_tiles = {}

            def issue_xt(j):
                xall = xtp.tile([128, 16 * 512], BF16, name="xall")
                if j < 2:
                    # per-tile loads so proj(j)'s d-loop starts asap
                    for d in range(16):
                        nc.sync.dma_start(
                            xall[:, d * 512:(d + 1) * 512],
                            xTt[j * 128:(j + 1) * 128,
                                d * 512:(d + 1) * 512])
                else:
                    # prefetched a pair ahead; 4 x 512KB so norm-chain
                    # DMAs queued behind never wait more than ~1.4us
                    for q in range(4):
                        nc.sync.dma_start(
                            xall[:, q * 2048:(q + 1) * 2048],
                            xTt[j * 128:(j + 1) * 128,
                                q * 2048:(q + 1) * 2048])
                xt_tiles[j] = xall
                if j == 0:
                    for d in range(16):
                        nc.scalar.dma_start(
                            wqkv_r[:, d * 384:(d + 1) * 384],
                            wqkvT[d * 128:(d + 1) * 128, :])
                    nc.scalar.dma_start(c4_sb[:], c4[:])
                    nc.scalar.dma_start(s4_sb[:], s4[:])
                elif j == 1:
                    nc.sync.dma_start(maskT_sb[:], maskT[:])
                    for t in range(2):
                        nc.sync.dma_start(woT_r[:, t * D:(t + 1) * D],
                                          woT[t * 128:(t + 1) * 128, :])

            issue_xt(0)

            # ---------------- projection + rope for one 512-token chunk
            def proj(j):
                xall = xt_tiles.pop(j)
                kb = rp.tile([64, 512], BF16, name="kb")
                vsb = rp.tile([64, 512], BF16, name="vsb")
                qrb = rp.tile([128, 512], BF16, name="qrb")
                qib = rp.tile([128, 512], BF16, name="qib")
                for ch in range(3):
                    ps = mmps.tile([128, 512], F32, name="mm")
                    for d in range(16):
                        nc.tensor.matmul(
                            ps[:],
                            wqkv_r[:, d * 384 + ch * 128:
                                   d * 384 + (ch + 1) * 128],
                            xall[:, d * 512:(d + 1) * 512],
                            start=(d == 0), stop=(d == 15))
                    # stage before the next tile() call recycles the slot
                    if ch == 0:
                        nc.scalar.copy(qrb[:], ps[:])
                    elif ch == 1:
                        nc.scalar.copy(qib[:], ps[:])
                    else:
                        nc.scalar.copy(kb[:], ps[0:64, :])
                        nc.scalar.copy(vsb[:], ps[64:128, :])
                    drain_wo(1)
                # V transpose tiles (PE, psum slot shared with proj/wo)
                vtp = mmps.tile([128, 256], BF16, name="vtp", tag="mm")
                for i in range(4):
                    nc.tensor.transpose(
                        vtp[:, i * 64:(i + 1) * 64],
                        vsb[:, i * 128:(i + 1) * 128], ident64[:])
                vtp3 = vtp.rearrange("p (k c) -> p k c", c=64)
                nc.vector.tensor_copy(
                    Vt3[:, j * 4:j * 4 + 4, 0:64], vtp3[:])
                drain_wo(1)

                def rope():
                    tb = j * 512
                    bc = (j % 4) * 512
                    cs = c4_sb[:, bc:bc + 512]
                    sn = s4_sb[:, bc:bc + 512]
                    cs32 = c4_sb[0:32, bc:bc + 512]
                    sn32 = s4_sb[0:32, bc:bc + 512]
                    u1 = rp.tile([32, 512], BF16, name="u1", bufs=1)
                    u2 = rp.tile([32, 512], BF16, name="u2", bufs=1)
                    u3 = rp.tile([32, 512], BF16, name="u3", bufs=1)
                    u4 = rp.tile([32, 512], BF16, name="u4", bufs=1)
                    cs32b = c4_sb[32:64, bc:bc + 512]
                    sn32b = s4_sb[32:64, bc:bc + 512]
                    nc.vector.tensor_mul(u1[:], kb[0:32, :], cs32)
                    nc.vector.tensor_mul(u2[:], kb[32:64, :], sn32b)
                    nc.vector.tensor_mul(u3[:], kb[0:32, :], sn32)
                    nc.vector.tensor_mul(u4[:], kb[32:64, :], cs32b)
                    for g in (0, 64):
                        nc.vector.tensor_sub(
                            KRI2[g:g + 32, tb:tb + 512], u1[:], u2[:])
                    for g in (32, 96):
                        nc.vector.tensor_add(
                            KRI2[g:g + 32, tb:tb + 512], u3[:], u4[:])
                    t1 = rp.tile([128, 512], BF16, name="t1", bufs=1)
                    t2 = rp.tile([128, 512], BF16, name="t2", bufs=1)
                    t3 = rp.tile([128, 512], BF16, name="t3", bufs=1)
                    t4 = rp.tile([128, 512], BF16, name="t4", bufs=1)
                    nc.vector.tensor_mul(t1[:], qrb[:], cs)
                    nc.vector.tensor_mul(t3[:], qrb[:], sn)
                    nc.vector.tensor_mul(t2[:], qib[:], sn)
                    nc.vector.tensor_mul(t4[:], qib[:], cs)
                    for hh in range(4):
                        dst = QRI_A if hh < 2 else QRI_B
                        base = (hh % 2) * 64
                        nc.vector.tensor_sub(
                            dst[base:base + 32, tb:tb + 512],
                            t1[32 * hh:32 * hh + 32, :],
                            t2[32 * hh:32 * hh + 32, :])
                        nc.vector.tensor_add(
                            dst[base + 32:base + 64, tb:tb + 512],
                            t3[32 * hh:32 * hh + 32, :],
                            t4[32 * hh:32 * hh + 32, :])
                return rope

            # ---------------- attention machinery
            pending = deque()
            norm_q = deque()
            nprog = {"enq": 0, "pi_done": 0}
            NPAIRS = 8

            def enqueue_wo(attn01, attn23, qb):
                state = {}
                pid = nprog["enq"]
                nprog["enq"] += 1
                for qs in range(4):
                    for do in range(4):
                        pending.append(
                            (pid, attn01, attn23, qb, qs, do, state))

            def drain_wo(n, reserve=0):
                for _ in range(n):
                    if len(pending) <= reserve:
                        return
                    if pending[0][0] >= nprog["pi_done"] // 2:
                        return   # this pair's attn not normed yet
                    pid, attn01, attn23, qb, qs, do, state = \
                        pending.popleft()
                    last = pid == NPAIRS - 1
                    if do == 0:
                        state[qs] = outp.tile([128, D], BF16, name="osb")
                    osb = state[qs]
                    Ops = mmps.tile([128, 512], F32, name="mm")
                    nc.tensor.matmul(
                        Ops[:], attn01[:, qs * 128:(qs + 1) * 128],
                        woT_r[:, do * 512:(do + 1) * 512],
                        start=True, stop=False)
                    nc.tensor.matmul(
                        Ops[:], attn23[:, qs * 128:(qs + 1) * 128],
                        woT_r[:, D + do * 512:D + (do + 1) * 512],
                        start=False, stop=True)
                    if last and do % 2 == 1:
                        nc.scalar.copy(
                            osb[:, do * 512:(do + 1) * 512], Ops[:])
                    else:
                        nc.vector.tensor_copy(
                            osb[:, do * 512:(do + 1) * 512], Ops[:])
                    if do == 3:
                        qq = qb + qs * 128
                        eng = nc.scalar if (last and qs % 2) else nc.sync
                        eng.dma_start(o[qq:qq + 128, :], osb[:])

            def emit_pv(PVs, b, pkt, ppg, pcsl, stop):
                vt = Vt3[:, b * 16 + pkt, :]
                st = (pkt == 0)
                for hh in range(2):
                    hs = hh * 512
                    nc.tensor.matmul(
                        PVs[:, hs + pcsl.start:hs + pcsl.stop], vt,
                        ppg[:, hs + pcsl.start:hs + pcsl.stop],
                        start=st, stop=stop)

            def make_norm(pvc, pi, attn, pe_bcst=False):
                st = {}
                cb = 1024 * pi

                def mul_stage(bcst):
                    for hh in range(2):
                        nc.vector.tensor_mul(
                            attn[64 * hh:64 * hh + 64, :],
                            pvc[0:64, cb + 512 * hh:cb + 512 * hh + 512],
                            bcst[:, 512 * hh:512 * hh + 512])
                    nprog["pi_done"] += 1

                if pe_bcst:
                    # tail chains: every DMA hop costs ~2us of completion
                    # receipt latency, so run Ln/Exp directly on the
                    # [1,1024] denominator row and broadcast on the
                    # (idle) PE via a K=1 bf16 outer product.
                    def t1():
                        st["l1"] = normp.tile([1, 1024], F32, name="l1")
                        nc.scalar.activation(st["l1"][:],
                                             pvc[64:65, cb:cb + 1024],
                                             AF.Ln)

                    def t2():
                        st["r1"] = normp.tile([1, 1024], BF16, name="r1")
                        nc.scalar.activation(st["r1"][:], st["l1"][:],
                                             AF.Exp, scale=-1.0)

                    def t3():
                        bcst = sps.tile([64, 1024], F32, name="Sg",
                                        tag="Sg")
                        for hh in range(2):
                            nc.tensor.matmul(
                                bcst[:, 512 * hh:512 * hh + 512],
                                ones64[:],
                                st["r1"][:, 512 * hh:512 * hh + 512],
                                start=True, stop=True)
                        mul_stage(bcst)
                    return [t1, t2, t3]

                def s1():
                    st["d8"] = normp.tile([8, 128], F32, name="d8")
                    nc.sync.dma_start(st["d8"][:],
                                      pvc[64:65, cb:cb + 1024])

                def s2():
                    l8 = normp.tile([8, 128], F32, name="l8")
                    nc.scalar.activation(l8[:], st["d8"][:], AF.Ln)
                    st["r8"] = normp.tile([8, 128], BF16, name="r8")
                    nc.scalar.activation(st["r8"][:], l8[:],
                                         AF.Exp, scale=-1.0)

                def s3():
                    rec1 = normp.tile([1, 1024], BF16, name="rec1")
                    nc.sync.dma_start(rec1[:], st["r8"][:])
                    bcst = normp.tile([64, 1024], BF16, name="bc")
                    nc.gpsimd.partition_broadcast(bcst[:], rec1[:])
                    mul_stage(bcst)
                return [s1, s2, s3]

            def attention(b, jp, rope_mid=None):
                qb = b * S + jp * 512
                nkt = 4 * jp + 4
                # during the final pair, hold back a few normed wo steps so
                # the tail's norm-latency window has PE work queued
                rsv = 6 if (b, jp) == (1, 3) else 0
                attn01 = attnp.tile([128, 512], BF16, name="at01")
                attn23 = attnp.tile([128, 512], BF16, name="at23")
                pvc = normp.tile([65, 2048], F32, name="pvc")
                QRIs = (QRI_A, QRI_B)
                attns = (attn01, attn23)
                pgqs = (deque(), deque())

                def emit_score(pi, kt):
                    kc = b * S + kt * 128
                    r = kt - (nkt - 4)
                    cs0 = max(0, 128 * r)   # live q-col start
                    Sg = sps.tile([128, 1024], F32, name="Sg")
                    pg = probsp.tile([128, 1024], BF16, name="pg")
                    for hh in range(2):
                        hs = hh * 512
                        nc.tensor.matmul(
                            Sg[:, hs + cs0:hs + 512],
                            KRI2[64 * hh:64 * hh + 64, kc:kc + 128],
                            QRIs[pi][64 * hh:64 * hh + 64,
                                     qb + cs0:qb + 512],
                            start=True, stop=True,
                            tile_position=(64 * hh, 0))
                    sgv = Sg.rearrange(
                        "p (h c) -> p h c", h=2)[:, :, cs0:512]
                    pgv = pg.rearrange(
                        "p (h c) -> p h c", h=2)[:, :, cs0:512]
                    nc.scalar.activation(pgv, sgv, AF.Exp, scale=0.125)
                    if r >= 0:
                        pgt = pg.rearrange(
                            "p (h c) -> p h c", h=2)[:, :, cs0:cs0 + 128]
                        nc.vector.tensor_mul(pgt, pgt, maskT3[:])
                    pgqs[pi].append((kt, pg, slice(cs0, 512)))

                def finish_pi(pi, PVs):
                    pgq = pgqs[pi]
                    while pgq:
                        item = pgq.popleft()
                        emit_pv(PVs, b, *item, stop=not pgq)
                        drain_wo(1, rsv)
                    if norm_q:
                        norm_q.popleft()()
                    # free the PV banks asap; norm (per pi) is deferred
                    # into the following kt loops
                    nc.vector.tensor_copy(
                        pvc[:, 1024 * pi:1024 * pi + 1024], PVs[:])
                    norm_q.extend(make_norm(pvc, pi, attns[pi],
                                            pe_bcst=rsv > 0))

                # ---- pi0 ----
                drain_wo(2, rsv)
                PVs0 = pvps.tile([65, 1024], F32, name="PV")
                for kt in range(nkt):
                    # drains before the score so their DVE casts never sit
                    # ahead of this kt's mask mul (which gates its PV)
                    drain_wo(2 if len(pending) > 8 else 1, rsv)
                    emit_score(0, kt)
                    if len(pgqs[0]) > 2:
                        emit_pv(PVs0, b, *pgqs[0].popleft(), stop=False)
                    if kt in (1, 2, 3) and norm_q:
                        norm_q.popleft()()
                # hoist pi1's first scores to cover pi0's exp tail
                emit_score(1, 0)
                drain_wo(1, rsv)
                emit_score(1, 1)
                finish_pi(0, PVs0)
                if rope_mid is not None:
                    rope_mid()
                # ---- pi1 ----
                PVs1 = pvps.tile([65, 1024], F32, name="PV")
                for kt in range(2, nkt):
                    drain_wo(2 if len(pending) > 8 else 1, rsv)
                    emit_score(1, kt)
                    if len(pgqs[1]) > 2:
                        emit_pv(PVs1, b, *pgqs[1].popleft(), stop=False)
                    if kt in (2, 3, 4) and norm_q:
                        norm_q.popleft()()
                finish_pi(1, PVs1)
                drain_wo(2, rsv)
                enqueue_wo(attn01, attn23, qb)

            # ---------------- fused emission stream
            stream = [("p", 0), ("p", 1), ("a", 0, 0), ("p", 2),
                      ("a", 0, 1), ("p", 3), ("a", 0, 2), ("p", 4),
                      ("a", 0, 3), ("p", 5), ("a", 1, 0), ("p", 6),
                      ("a", 1, 1), ("p", 7), ("a", 1, 2), ("a", 1, 3)]
            next_xt = 2
            rope_pend = None
            for step in stream:
                if step[0] == "p":
                    j = step[1]
                    rope_fn = proj(j)
                    if j < 2:
                        # needed by the very next pair; emit inline
                        rope_fn()
                    else:
                        rope_pend = rope_fn
                    if j == 0:
                        issue_xt(1)
                else:
                    if next_xt < 8:
                        issue_xt(next_xt)
                        next_xt += 1
                    # rope for chunk j is emitted mid-pair (between pi0
                    # and pi1) so it never sits ahead of the pair's norm
                    # muls or next pair's masks on the DVE queue
                    attention(step[1], step[2], rope_mid=rope_pend)
                    rope_pend = None
            # emit any ungated wo steps (the last pair's reserve) BEFORE
            # the final norm chain so the PE has work during its latency;
            # high_priority makes the list scheduler place them as early
            # as their deps allow instead of after the norm chain
            with tc.high_priority():
                drain_wo(1 << 30)
            while norm_q:
                norm_q.popleft()()
            drain_wo(1 << 30)

    nc.compile()
    bacc.get_activation_tables = _orig_gat
    return nc


def _prep_inputs(x, freqs_cos, freqs_sin, wq, wk, wv, wo):
    from ml_dtypes import bfloat16
    xf = np.asarray(x, np.float32).reshape(T, D)
    xTf = np.ascontiguousarray(xf.T).astype(bfloat16)      # [D, T]
    # j-major p-row-major layout: xTt[j, p, d, c] = xT[d*128+p, j*512+c]
    xTt = np.ascontiguousarray(
        xTf.reshape(16, 128, 8, 512).transpose(2, 1, 0, 3)
    ).reshape(8 * 128, 16 * 512)
    wq = np.asarray(wq, np.float32)
    wk = np.asarray(wk, np.float32)
    wv = np.asarray(wv, np.float32)
    wo = np.asarray(wo, np.float32)
    fc = np.asarray(freqs_cos, np.float32)
    fs = np.asarray(freqs_sin, np.float32)

    c4 = np.ascontiguousarray(np.tile(fc.T, (4, 1))).astype(bfloat16)
    s4 = np.ascontiguousarray(np.tile(fs.T, (4, 1))).astype(bfloat16)
    kt = np.arange(128)[:, None]
    qt = np.arange(128)[None, :]
    tri = (kt <= qt).astype(np.float32)
    maskT = np.ascontiguousarray(np.tile(tri, (1, 2))).astype(bfloat16)
    ev = np.arange(0, 64, 2)
    od = np.arange(1, 64, 2)

    in_maps = []
    for c in range(NCORES):
        qreal = np.concatenate([(4 * c + h) * 64 + ev for h in range(4)])
        qimag = np.concatenate([(4 * c + h) * 64 + od for h in range(4)])
        Wc = np.concatenate([wq[qreal], wq[qimag], wk[c * 64 + ev],
                             wk[c * 64 + od], wv[c * 64:(c + 1) * 64]], axis=0)
        in_maps.append({
            "xTt": xTt,
            "wqkvT": np.ascontiguousarray(Wc.T).astype(bfloat16),
            "woT": np.ascontiguousarray(
                wo[:, c * 256:(c + 1) * 256].T).astype(bfloat16),
            "c4": c4, "s4": s4, "maskT": maskT,
        })
    return in_maps


def _run(in_maps, trace=False, **kw):
    from concourse import bass_utils
    if "nc" not in _cache:
        _cache["nc"] = _build()
    return bass_utils.run_bass_kernel_spmd(
        _cache["nc"], in_maps, core_ids=list(range(NCORES)), trace=trace, **kw)


def kernel(x, freqs_cos, freqs_sin, wq, wk, wv, wo):
    in_maps = _prep_inputs(x, freqs_cos, freqs_sin, wq, wk, wv, wo)
    res = _run(in_maps)
    out = np.zeros((T, D), np.float64)
    for c in range(NCORES):
        out += np.asarray(res.results[c]["o"], np.float32)
    return out.astype(np.float32).reshape(B, S, D)
